# revision 32
# baseline (speedup 1.0000x reference)
"""Local causal (sliding-window) attention kernel for Trainium2, SPMD over 8 NeuronCores.

Problem: x [1,4096,1024] -> QKV proj -> 16-head attention with causal window 64
         -> out proj. All fp32 at the interface.

Sharding: sequence-parallel. Core c owns queries [512c, 512c+512). Attention is
local (window 64), so each core only needs a 128-row key/value halo (the
previous 128-token block) in addition to its own 512 rows. Each core computes
its full output rows; host concatenates. No collectives.

All on-chip compute is bf16 (fp32 PSUM accumulation): host casts x/weights to
bf16, which halves HBM traffic (the projection phase is DMA-paced) and runs
every matmul at 1 cycle/row regardless of free-dim size. Measured end-to-end
rel err vs the fp32 reference is ~6e-3.

Attention is computed TRANSPOSED (S^T = K^T-stationary x Q): exp(S^T) is
already P^T, so no PE transposes and no P^T staging copies are needed. V tiles
carry an interleaved ones-block per head ([V_h | 1]), so each PV matmul also
accumulates the softmax denominators into psum rows 64:128 for free; the
normalization (x 1/denom) is fused into the psum->sbuf copy of the attention
output (DVE reciprocal + tensor_mul).

Per-core layouts (host pre-transposes so every DMA is a clean row-major tile):
  xT    [1024 d, 640 n]   x^T for rows [s-128, s+512) (core 0: first 128 zero)
  wqkvT [1024 d, 3072 o]  w_qkv^T
  woutT [1024 d, 1024 o]  w_out^T
  maskT [128, 768]        0/1 valid bits for the 5 S^T strips of one head
                          (per-core data; core 0 zeroes the kb0 strip)

S^T strips are trimmed to the query range that can see the strip's keys
(STRIP_Q, 768 columns total); psum banks A=[kb0, kb1, kb4], B=[kb2, kb3];
the sbuf P^T tile is [A | B] = [128, 768]. Strip kb holds keys
k = 128kb + row; entry (row, q) is valid iff q+65 <= k <= q+128.
"""

from contextlib import ExitStack

import ml_dtypes
import numpy as np

import concourse.bass as bass
import concourse.mybir as mybir
import concourse.tile as tile
from concourse import bacc
from concourse.bass_utils import run_bass_kernel_spmd

F32 = mybir.dt.float32
BF16 = mybir.dt.bfloat16
FP8 = mybir.dt.float8e4
NP_BF16 = ml_dtypes.bfloat16
NP_FP8 = ml_dtypes.float8_e4m3
DR = mybir.MatmulPerfMode.DoubleRow


D_MODEL = 1024
N_HEADS = 16
D_HEAD = 64
WINDOW = 64
N_SEQ = 4096
N_CORES = 8
NLOC = N_SEQ // N_CORES          # 512 queries per core
HALO = 128                       # one full key block of halo
NTOT = NLOC + HALO               # 640 local rows (keys/values)
QB = 128                         # query block
NQB = NLOC // QB                 # 4 query blocks per core
KB = 128                         # key block
NKB = NTOT // KB                 # 5 key blocks per core
SCALE = 1.0 / np.sqrt(D_HEAD)

DT = D_MODEL // 128              # 8 contraction tiles

# Diagonal 64-query strips: strip u covers queries [64u, 64u+64) whose full
# key windows [q+65, q+128] all fall inside keys [64(u+1), 64(u+1)+128) --
# exactly 128 keys, so every (head, u) score/PV is ONE matmul with no
# cross-strip accumulation. 8 strips x 64 cols = 512 P^T columns per head
# (vs 768 for 128-key-block strips). Entry (k', q') of strip u is valid iff
# 1 <= k' - q' <= 64 -- the same [128, 64] band mask for every strip.
NSTRIP = 8
STRIP_COLS = 512
# maskT layout: cols 0:512 = the 8 strips' band masks; cols 512:640 = a
# 128x128 identity used by the PE transpose in the V-halo path.
MASKT_COLS = STRIP_COLS + 128

# exposed for test.py (profiling info)
LAST_RESULT = None


def _emit_program(use_bqkv: bool, use_bout: bool, reps: int = 1) -> bass.Bass:
    # Bacc (not raw Bass): its finalize pipeline splits semaphore waits
    # (move_matmul_waits_to_ldweights / generate_event_semaphores) to satisfy
    # the HW limit of 1 sync wait per instruction.
    nc = bacc.Bacc()

    # fp8 hi/lo split operands (see _split8): A = term1 pairs (hi chunk 2j,
    # hi chunk 2j+1), B = cross-term pairs (x: (lo16, hi16); w: (hi16, lo16)).
    xA = nc.declare_dram_parameter("xA", [D_MODEL // 2, 2 * NTOT], FP8, isOutput=False)
    xB = nc.declare_dram_parameter("xB", [D_MODEL, 2 * NTOT], FP8, isOutput=False)
    wqkvA = nc.declare_dram_parameter("wqkvA", [D_MODEL // 2, 3 * 2 * D_MODEL], FP8, isOutput=False)
    wqkvB = nc.declare_dram_parameter("wqkvB", [D_MODEL, 3 * 2 * D_MODEL], FP8, isOutput=False)
    woutT = nc.declare_dram_parameter("woutT", [D_MODEL, D_MODEL], BF16, isOutput=False)
    maskT = nc.declare_dram_parameter("maskT", [128, MASKT_COLS], BF16, isOutput=False)
    if use_bqkv:
        bqkv = nc.declare_dram_parameter("bqkv", [1, 3 * D_MODEL], BF16, isOutput=False)
    if use_bout:
        bout = nc.declare_dram_parameter("bout", [1, D_MODEL], BF16, isOutput=False)
    y = nc.declare_dram_parameter("y", [NLOC, D_MODEL], BF16, isOutput=True)

    with tile.TileContext(nc) as tc:
      for _rep in range(reps):
       with ExitStack() as ctx:
        consts = ctx.enter_context(tc.tile_pool(name="consts", bufs=1))
        xpool = ctx.enter_context(tc.tile_pool(name="xpool", bufs=1))
        wpool = ctx.enter_context(tc.tile_pool(name="wpool", bufs=44))
        qtp = ctx.enter_context(tc.tile_pool(name="qtp", bufs=1))
        ktp = ctx.enter_context(tc.tile_pool(name="ktp", bufs=1))
        vp = ctx.enter_context(tc.tile_pool(name="vp", bufs=1))
        aop = ctx.enter_context(tc.tile_pool(name="aop", bufs=1))
        work = ctx.enter_context(tc.tile_pool(name="work", bufs=2))
        outp = ctx.enter_context(tc.tile_pool(name="outp", bufs=6))
        # PSUM: four role-dedicated tags x 2 slots (1 bank each). Each tag's
        # tiles are read by exactly one engine, keeping matmul wait counts <=2.
        psum = ctx.enter_context(tc.tile_pool(name="psum", bufs=2, space="PSUM"))

        # psum->sbuf copies get EXPLICIT engines: every psum tag must have a
        # single reader engine so a consumer matmul's waits stay within the
        # HW limit of 2 sync-wait commands (producer sem + WAR sem).
        def copy_act(dst, src):
            nc.scalar.copy(dst, src)

        def copy_dve(dst, src):
            nc.vector.tensor_copy(dst, src)

        # During the DMA-paced projection phases the attention psum tags are
        # idle; rotating projection psums across all four tags gives 8 chains
        # in flight instead of 2 (Bacc splits any extra semaphore waits).
        ps_rot = ["ps", "s", "pt", "pA"]
        ps_idx = [0]

        def next_ps(cols, nm):
            tag = ps_rot[ps_idx[0] % 4]
            ps_idx[0] += 1
            return psum.tile([128, cols], F32, tag=tag, name=nm, bufs=2)

        # ---- load x^T (8 tiles [128, 640]) interleaved with wv so the V
        # accumulation chains can start as soon as the first pair lands ----
        # PE pstate ramp: the tensor engine reaches full clock 3us after its
        # first instruction. Fire a trivial matmul on locally-memset data
        # immediately (no DMA dependency) so the ramp clock starts at ~t=0.3us
        # instead of ~2.4us when the first loads land (~1us saved).
        zt = consts.tile([1, 8], BF16, tag="zt")
        nc.gpsimd.memset(zt, 0.0)
        warm_ps = psum.tile([1, 8], F32, tag="pt", name="warm", bufs=2)
        nc.tensor.matmul(warm_ps, lhsT=zt[0:1, 0:1], rhs=zt[0:1, 0:8],
                         start=True, stop=True)

        # All projection chains run fp8 DoubleRow (0.5 cyc/row, 256-wide
        # contraction): out = x_hi w_hi (4 A-pair matmuls) + x_lo w_hi +
        # x_hi w_lo (8 B-pair matmuls) -- bf16-level accuracy at 0.75x the
        # bf16 PE cost. The dropped lo*lo term is ~2^-8 relative.
        xtA = [xpool.tile([128, 2 * NTOT], FP8, tag=f"xA{j}", name=f"xA{j}")
               for j in range(DT // 2)]
        xtB = [xpool.tile([128, 2 * NTOT], FP8, tag=f"xB{g}", name=f"xB{g}")
               for g in range(DT)]
        wvA = [wpool.tile([128, 2 * D_MODEL], FP8, tag="w", name=f"wvA{j}")
               for j in range(DT // 2)]
        wvB = [wpool.tile([128, 2 * D_MODEL], FP8, tag="w", name=f"wvB{g}")
               for g in range(DT)]
        WVC = 2 * 2 * D_MODEL            # wqkvA/B column offset of the V proj
        for j in range(DT // 2):
            nc.sync.dma_start(out=xtA[j], in_=xA[j * 128:(j + 1) * 128, :])
            # wv rides the idle Pool queue so it transfers in parallel with
            # the x stream and the first V matmul can start ~500ns earlier
            nc.gpsimd.dma_start(out=wvA[j], in_=wqkvA[j * 128:(j + 1) * 128, WVC:WVC + 2 * D_MODEL])
        for g in range(DT):
            # split the xB stream across both queues so SP frees up earlier
            # for the wq stream (Q chains are the first QK-phase consumers)
            eng = nc.sync if g < DT // 2 else nc.gpsimd
            eng.dma_start(out=xtB[g], in_=xB[g * 128:(g + 1) * 128, :])
            nc.gpsimd.dma_start(out=wvB[g], in_=wqkvB[g * 128:(g + 1) * 128, WVC:WVC + 2 * D_MODEL])

        def pr2(t):
            # pair view [128, 2, C] of an interleaved-pair tile [128, 2C]
            return t[:, :].rearrange("p (two c) -> p two c", two=2)

        def fp8_chain(ps, ocols, xcols, wA, wB, wc0, wcols, w_stationary, stop_last):
            # 12-matmul DoubleRow chain contracting 1024: 4 A-pairs + 8 B-pairs.
            # w_stationary: lhsT = w pairs (Q/K orientation), else lhsT = x pairs.
            for j in range(DT // 2):
                xap = pr2(xtA[j])[:, :, xcols[0]:xcols[1]]
                wap = pr2(wA[j])[:, :, wc0:wc0 + wcols]
                nc.tensor.matmul(
                    ps, lhsT=wap if w_stationary else xap,
                    rhs=xap if w_stationary else wap,
                    start=(j == 0), stop=False, perf_mode=DR)
            for g in range(DT):
                xbp = pr2(xtB[g])[:, :, xcols[0]:xcols[1]]
                wbp = pr2(wB[g])[:, :, wc0:wc0 + wcols]
                nc.tensor.matmul(
                    ps, lhsT=wbp if w_stationary else xbp,
                    rhs=xbp if w_stationary else wbp,
                    start=False, stop=(g == DT - 1 and stop_last), perf_mode=DR)

        # ---- constants, queued AFTER the x/wv stream (not needed until the
        # attention phase; keeping them off the head of the DMA queue lets PE
        # start ~1us earlier) ----
        mT = consts.tile([128, MASKT_COLS], BF16, tag="mT")
        nc.sync.dma_start(out=mT, in_=maskT[:, :])
        if use_bqkv or use_bout:
            ones = consts.tile([1, 512], BF16, tag="ones")
            nc.vector.memset(ones, 1.0)
        if use_bqkv:
            bqkv_sb = consts.tile([1, 3 * D_MODEL], BF16, tag="bqkv")
            nc.sync.dma_start(out=bqkv_sb, in_=bqkv[:, :])
        if use_bout:
            bout_sb = consts.tile([1, D_MODEL], BF16, tag="bout")
            nc.sync.dma_start(out=bout_sb, in_=bout[:, :])

        # ---- Phase V: V'[n, h*128+(0:64)] = (x @ wv^T)_h, V'[n, h*128+(64:128)] = 1
        # The interleaved ones-blocks make every PV matmul accumulate the
        # softmax denominators into psum rows 64:128 at zero PE cost. ----
        vt = [None]
        for n in range(1, NKB):
            t = vp.tile([128, N_HEADS * 128], BF16, tag=f"v{n}", name=f"v{n}")
            onesview = t[:, :].rearrange("p (h c) -> p h c", c=128)[:, :, D_HEAD:128]
            nc.vector.memset(onesview, 1.0)
            vt.append(t)
        for n in range(1, NKB):
            for oh in range(2):
                ps = next_ps(512, "psv")
                fp8_chain(ps, 512, (n * 128, (n + 1) * 128), wvA, wvB,
                          oh * 512, 512, w_stationary=False,
                          stop_last=not use_bqkv)
                if use_bqkv:
                    nc.tensor.matmul(
                        ps, lhsT=ones[0:1, 0:128],
                        rhs=bqkv_sb[0:1, 2 * D_MODEL + oh * 512:2 * D_MODEL + (oh + 1) * 512],
                        start=False, stop=True)
                # strided copy: head j of this half -> V' block (8*oh+j)*128
                dst = vt[n][:, oh * 1024:(oh + 1) * 1024].rearrange(
                    "p (h c) -> p h c", c=128)[:, :, 0:D_HEAD]
                src = ps[:, :].rearrange("p (h c) -> p h c", c=D_HEAD)
                copy_dve(dst, src)
        # Even-u PV strips need V' rows at 64-skewed offsets (64+128j : 192+128j),
        # which straddle two A-tiles; build skewed B-tiles by plain partition-
        # shifted SBUF copies (ones blocks come along for free).
        vtB = []
        for j in range(NKB - 1):
            t = vp.tile([128, N_HEADS * 128], BF16, tag=f"vB{j}", name=f"vB{j}")
            # DVE is free between the V-phase psum copies and the first PV
            # normalize; Pool's queue is needed for the wv/wk DMA streams.
            if j > 0:
                nc.vector.tensor_copy(t[0:64, :], vt[j][64:128, :])
            nc.vector.tensor_copy(t[64:128, :], vt[j + 1][0:64, :])
            vtB.append(t)
        # V-halo (rows 64:128) -> vtB[0] top half, odim-major + PE transpose.
        # Block 0's keys-major chain would cost the full 2x8x512 F for 64 live
        # rows; instead each odim-tile (2 heads) is an 8-matmul F=64 chain into
        # psum [128 odims, 64 keys], staged to sbuf, transposed on PE via the
        # identity in mT, and strided-copied into the two heads' value blocks
        # (5120 cycles instead of 8192).
        onesview0 = vtB[0][0:64, :].rearrange("p (h c) -> p h c", c=128)[:, :, D_HEAD:128]
        nc.vector.memset(onesview0, 1.0)
        for t8 in range(DT):
            hv = psum.tile([128, 64], F32, tag="s", name="hv", bufs=2)
            fp8_chain(hv, 64, (64, 128), wvA, wvB,
                      t8 * 128, 128, w_stationary=True, stop_last=True)
            hv_sb = work.tile([128, 64], BF16, tag="hvs", bufs=2, name="hvs")
            copy_act(hv_sb, hv)
            hv_t = psum.tile([64, 128], BF16, tag="pA", name="hvt", bufs=2)
            nc.tensor.transpose(hv_t, hv_sb, mT[:, STRIP_COLS:STRIP_COLS + 128])
            dst = vtB[0][0:64, t8 * 256:(t8 + 1) * 256].rearrange(
                "p (h c) -> p h c", c=128)[:, :, 0:D_HEAD]
            src = hv_t[:, :].rearrange("p (h c) -> p h c", c=D_HEAD)
            copy_dve(dst, src)

        # ---- Phase Q/K + attention, software-pipelined ----
        # Head pairs are processed in order [1..6 in-loop, then 7, then 0]:
        # the LAST pair processed (0) uses qt/kt tiles ready since o=0, so the
        # tail never waits on fresh projection copies; the out-proj chains
        # contract g=0 last for the same reason.
        wqA, wqB, wkA, wkB = [], [], [], []
        for j in range(DT // 2):
            t = wpool.tile([128, 2 * D_MODEL], FP8, tag="w", name=f"wqA{j}")
            nc.sync.dma_start(out=t, in_=wqkvA[j * 128:(j + 1) * 128, 0:2 * D_MODEL])
            wqA.append(t)
            t = wpool.tile([128, 2 * D_MODEL], FP8, tag="w", name=f"wkA{j}")
            nc.sync.dma_start(out=t, in_=wqkvA[j * 128:(j + 1) * 128, 2 * D_MODEL:4 * D_MODEL])
            wkA.append(t)
        for g in range(DT):
            t = wpool.tile([128, 2 * D_MODEL], FP8, tag="w", name=f"wqB{g}")
            nc.sync.dma_start(out=t, in_=wqkvB[g * 128:(g + 1) * 128, 0:2 * D_MODEL])
            wqB.append(t)
            t = wpool.tile([128, 2 * D_MODEL], FP8, tag="w", name=f"wkB{g}")
            nc.gpsimd.dma_start(out=t, in_=wqkvB[g * 128:(g + 1) * 128, 2 * D_MODEL:4 * D_MODEL])
            wkB.append(t)

        wo = []
        for g in range(DT):
            t = wpool.tile([128, D_MODEL], BF16, tag="w", name=f"wo{g}")
            nc.sync.dma_start(out=t, in_=woutT[g * 128:(g + 1) * 128, :])
            wo.append(t)

        qt = [qtp.tile([128, NLOC], BF16, tag=f"qt{o}", name=f"qt{o}") for o in range(DT)]
        kt = [ktp.tile([128, NTOT], BF16, tag=f"kt{o}", name=f"kt{o}") for o in range(DT)]
        # Keys 0:64 can never be attended (query q sees keys >= q+65) and no
        # diagonal strip reads them (strip u starts at key 64(u+1) >= 64), so
        # kt cols 0:64 are simply never written.
        ao = [aop.tile([128, NLOC], BF16, tag=f"ao{g}", name=f"ao{g}") for g in range(DT)]

        def emit_qk(o, split_copies=False):
            # QT o-tile: out [128 o, 512 n]; rhs = own rows = xT cols [128, 640)
            # Exp and Copy share an ACT function-set table (act_info.json:
            # exp_and_others), so alternating them costs no table reloads
            cp = copy_act

            def copy_out(dst, src):
                if split_copies:
                    # halve the copies so head 2o's scores (rows 0:64) can
                    # issue after the first half lands (shortens the tail)
                    cp(dst[0:64], src[0:64])
                    cp(dst[64:128], src[64:128])
                else:
                    cp(dst, src)

            ps = next_ps(512, "psq")
            fp8_chain(ps, 512, (HALO, NTOT), wqA, wqB,
                      o * 128, 128, w_stationary=True,
                      stop_last=not use_bqkv)
            if use_bqkv:
                nc.tensor.matmul(
                    ps, lhsT=bqkv_sb[0:1, o * 128:(o + 1) * 128],
                    rhs=ones[0:1, 0:512], start=False, stop=True)
            copy_out(qt[o], ps)
            # KT o-tile: rows 64:640 (dead halo cols skipped), two N=288 chains
            for (c0, cw) in ((64, 288), (352, 288)):
                ps = next_ps(cw, "pskt")
                fp8_chain(ps[:, 0:cw], cw, (c0, c0 + cw), wkA, wkB,
                          o * 128, 128, w_stationary=True,
                          stop_last=not use_bqkv)
                if use_bqkv:
                    nc.tensor.matmul(
                        ps[:, 0:cw], lhsT=bqkv_sb[0:1, D_MODEL + o * 128:D_MODEL + (o + 1) * 128],
                        rhs=ones[0:1, 0:cw], start=False, stop=True)
                copy_out(kt[o][:, c0:c0 + cw], ps[:, 0:cw])

        head_state = {}

        def emit_head_scores(h):
            g = h // 2
            r0 = (h % 2) * D_HEAD          # row offset of head h inside tile g
            # S^T diagonal strips into ONE psum bank [128, 512]. The first
            # matmul carries start=True (marks the whole bank pending), later
            # ones first-touch-overwrite their regions, the last carries stop.
            s_ps = psum.tile([128, STRIP_COLS], F32, tag="s", name="sS", bufs=2)
            mm = nc.tensor.matmul
            for u in range(NSTRIP):
                mm(s_ps[:, 64 * u:64 * u + 64],
                   lhsT=kt[g][r0:r0 + D_HEAD, 64 * (u + 1):64 * (u + 1) + 128],
                   rhs=qt[g][r0:r0 + D_HEAD, 64 * u:64 * u + 64],
                   start=(u == 0), stop=(u == NSTRIP - 1), skip_group_check=True)
            # P^T = exp(SCALE * S^T); invalid entries hold finite junk
            # (|SCALE*s| <~ 12, no bf16 overflow), zeroed by the mask below.
            pt_t = work.tile([128, STRIP_COLS], BF16, tag="p", bufs=6, name=f"pt{h}")
            nc.scalar.activation(pt_t[:, 0:256], s_ps[:, 0:256],
                                 mybir.ActivationFunctionType.Exp,
                                 bias=0.0, scale=float(SCALE))
            nc.scalar.activation(pt_t[:, 256:512], s_ps[:, 256:512],
                                 mybir.ActivationFunctionType.Exp,
                                 bias=0.0, scale=float(SCALE))
            # zero the out-of-band entries (Pool; otherwise idle here).
            # Two halves, each pipelined behind its exp, to shorten the
            # exp->mask->PV round trip.
            nc.gpsimd.tensor_mul(pt_t[:, 0:256], pt_t[:, 0:256], mT[:, 0:256])
            nc.gpsimd.tensor_mul(pt_t[:, 256:512], pt_t[:, 256:512], mT[:, 256:512])
            head_state[h] = pt_t

        def emit_head_pv(h):
            g = h // 2
            r0 = (h % 2) * D_HEAD
            pt_t = head_state.pop(h)
            # out'_h [128, 512 q]: rows 0:64 = out_h^T, rows 64:128 = softmax
            # denominators (from the V' ones-blocks). Each strip u is a single
            # matmul over its 128-key window: odd u hits an aligned A-tile,
            # even u the 64-skewed B-tile.
            op = psum.tile([128, NLOC], F32, tag="pA", name="opsum", bufs=2)
            mm = nc.tensor.matmul
            for u in range(NSTRIP):
                vtile = vt[(u + 1) // 2] if u % 2 == 1 else vtB[u // 2]
                mm(op[:, 64 * u:64 * u + 64],
                   lhsT=vtile[:, h * 128:(h + 1) * 128],
                   rhs=pt_t[:, 64 * u:64 * u + 64],
                   start=(u == 0), stop=(u == NSTRIP - 1), skip_group_check=True)
            # normalize fused into the psum->sbuf copy: ao = out * (1/denom)
            # (DVE divide is rejected by the BIR verifier - no divide ALU;
            # Pool cannot read PSUM; ACT-copy decoupling adds a second psum
            # reader engine whose WAR semaphores cost more than it saves)
            rbb = work.tile([D_HEAD, NLOC], F32, tag="rbb", bufs=2, name="rbb")
            nc.vector.reciprocal(rbb, op[D_HEAD:128, :])
            nc.vector.tensor_mul(ao[g][r0:r0 + D_HEAD, :], op[0:D_HEAD, :], rbb)

        for o in range(DT):
            emit_qk(o)
            if o >= 3:
                emit_head_pv(2 * (o - 2))
                emit_head_pv(2 * (o - 2) + 1)
            if o >= 2:
                emit_head_scores(2 * (o - 1))
                emit_head_scores(2 * (o - 1) + 1)
        # tail: pair 0 (ancient tiles) and pair 7; PE filler (pv 12/13, dmy)
        # covers the exp->mask round trips of the last-scored pairs.
        emit_head_scores(0)
        emit_head_scores(1)
        emit_head_scores(14)
        emit_head_scores(15)
        emit_head_pv(12)
        emit_head_pv(13)
        # 8 trivial matmuls make PE observe every wo DMA queue semaphore
        # here (satisfied by now - wo was prefetched), so phase C's matmuls
        # don't each need a DMA wait slot (HW limit: 2 sync waits per matmul)
        dmy = psum.tile([1, 1], F32, tag="pt", name="dmy", bufs=2)
        for g in range(DT):
            nc.tensor.matmul(dmy, lhsT=wo[g][0:1, 0:1],
                             rhs=wo[g][0:1, 0:1],
                             start=(g == 0), stop=(g == DT - 1))
        emit_head_pv(0)
        emit_head_pv(1)
        emit_head_pv(14)
        emit_head_pv(15)

        # ---- Phase C: out = attnout @ wout^T (+ b_out); g=0 contracted last
        # so the chains only need ao[0] (heads 0/1, finishing on DVE) at the
        # very end of each chain ----
        gorder = [1, 2, 3, 4, 5, 6, 7, 0]

        chain_idx = [0]

        def outproj_chain(n, c0, cw, ps=None, cp=None, dma_eng=None):
            if ps is None:
                ps = next_ps(cw, "psc")
            if cp is None:
                cp = copy_dve
            for gi, g in enumerate(gorder):
                nc.tensor.matmul(
                    ps, lhsT=ao[g][:, n * 128:(n + 1) * 128],
                    rhs=wo[g][:, c0:c0 + cw],
                    start=(gi == 0), stop=(gi == DT - 1 and not use_bout))
            if use_bout:
                nc.tensor.matmul(
                    ps, lhsT=ones[0:1, 0:128],
                    rhs=bout_sb[0:1, c0:c0 + cw],
                    start=False, stop=True)
            # y is written in bf16 (host casts back to fp32): halves the DMA
            # bytes and doubles the DVE copy rate on the drain path.
            ot = outp.tile([128, cw], BF16, tag="out", name=f"ot{cw}")
            cp(ot, ps)
            if dma_eng is None:
                # alternate the y DMAs between the SP and Pool queues so the
                # drain's transfers overlap instead of serializing on SP
                dma_eng = nc.sync if chain_idx[0] % 2 == 0 else nc.gpsimd
            chain_idx[0] += 1
            dma_eng.dma_start(out=y[n * 128:(n + 1) * 128, c0:c0 + cw], in_=ot)

        for n in range(NQB):
            for oh in range(2):
                if n == NQB - 1 and oh == 1:
                    # drain choreography: the 384's DVE copy + ACT-queue DMA
                    # and the final 128's ACT copy (psum tag "s" is ACT-read)
                    # + SP DMA run on disjoint engine/queue pairs, so the two
                    # last completions overlap instead of serializing.
                    outproj_chain(n, 512, 384, dma_eng=nc.scalar)
                    fps = psum.tile([128, 128], F32, tag="s", name="fin", bufs=2)
                    outproj_chain(n, 896, 128, ps=fps, cp=copy_act,
                                  dma_eng=nc.sync)
                else:
                    outproj_chain(n, oh * 512, 512)

    return nc


_PROG_CACHE: dict = {}


def _get_program(use_bqkv: bool, use_bout: bool) -> bass.Bass:
    key = (use_bqkv, use_bout)
    if key not in _PROG_CACHE:
        nc = _emit_program(use_bqkv, use_bout)
        if not nc.is_finalized():
            nc.finalize()
        _PROG_CACHE[key] = nc
    return _PROG_CACHE[key]


def _build_maskT(core0: bool) -> np.ndarray:
    """0/1 valid bits for the 8 diagonal S^T strips, [128 k-rows, 512 cols].

    Strip u holds keys 64(u+1)+k' vs queries 64u+q'; valid iff
    1 <= k'-q' <= 64. On core 0 the u=0 strip's keys 64:128 (k' < 64) are
    x-padding, so they are masked off too."""
    m = np.zeros((128, MASKT_COLS), np.float32)
    kp = np.arange(128)[:, None]
    qp = np.arange(64)[None, :]
    band = (kp - qp >= 1) & (kp - qp <= WINDOW)
    for u in range(NSTRIP):
        v = band if not (core0 and u == 0) else band & (kp >= 64)
        m[:, 64 * u:64 * u + 64] = v
    m[:, STRIP_COLS:STRIP_COLS + 128] = np.eye(128, dtype=np.float32)
    return m


def _split8(a):
    """fp8 hi/lo split of an fp32 array: a ~= hi + lo16/16; hi16 = hi/16.

    The 16x scalings keep the lo and scaled-hi values inside e4m3's normal
    range so the cross terms (x_lo w_hi + x_hi w_lo) come out exactly scaled:
    x_lo16 . w_hi16 = x_lo . w_hi, etc."""
    hi = a.astype(NP_FP8)
    hif = hi.astype(np.float32)
    lo16 = ((a - hif) * 16.0).astype(NP_FP8)
    hi16 = (hif / 16.0).astype(NP_FP8)
    return hi, lo16, hi16


def _pairA(hT):
    """[1024, C] -> [512, 2C]: row 128j+p = [chunk(2j) row p | chunk(2j+1) row p]."""
    C = hT.shape[1]
    t = hT.reshape(4, 2, 128, C).transpose(0, 2, 1, 3)
    return np.ascontiguousarray(t.reshape(512, 2 * C))


def _pairB(first, second):
    """two [1024, C] -> [1024, 2C]: row 128g+p = [first(g) row p | second(g) row p]."""
    C = first.shape[1]
    t = np.stack([first.reshape(8, 128, C), second.reshape(8, 128, C)], axis=2)
    return np.ascontiguousarray(t.reshape(1024, 2 * C))


def kernel(x, w_qkv, b_qkv, w_out, b_out):
    global LAST_RESULT
    x = np.asarray(x, dtype=np.float32)
    w_qkv = np.asarray(w_qkv, dtype=np.float32)
    b_qkv = np.asarray(b_qkv, dtype=np.float32)
    w_out = np.asarray(w_out, dtype=np.float32)
    b_out = np.asarray(b_out, dtype=np.float32)

    B = x.shape[0]
    assert x.shape == (1, N_SEQ, D_MODEL), x.shape
    xa = x[0]

    use_bqkv = bool(np.any(b_qkv))
    use_bout = bool(np.any(b_out))
    nc = _get_program(use_bqkv, use_bout)

    wqkvT = np.ascontiguousarray(w_qkv.T)                    # [1024, 3072] f32
    w_hi, w_lo16, w_hi16 = _split8(wqkvT)
    wqkvA = np.concatenate(
        [_pairA(w_hi[:, p * D_MODEL:(p + 1) * D_MODEL]) for p in range(3)], axis=1)
    wqkvB = np.concatenate(
        [_pairB(w_hi16[:, p * D_MODEL:(p + 1) * D_MODEL],
                w_lo16[:, p * D_MODEL:(p + 1) * D_MODEL]) for p in range(3)], axis=1)
    woutT = np.ascontiguousarray(w_out.T).astype(NP_BF16)    # [1024, 1024]
    mT_std = _build_maskT(core0=False).astype(NP_BF16)
    mT_first = _build_maskT(core0=True).astype(NP_BF16)

    in_maps = []
    for c in range(N_CORES):
        s = c * NLOC
        if c == 0:
            blk = np.concatenate([np.zeros((HALO, D_MODEL), np.float32), xa[0:NLOC]], axis=0)
        else:
            blk = xa[s - HALO:s + NLOC]
        xTblk = np.ascontiguousarray(blk.T)                 # [1024, 640] f32
        x_hi, x_lo16, x_hi16 = _split8(xTblk)
        im = {
            "xA": _pairA(x_hi),
            "xB": _pairB(x_lo16, x_hi16),
            "wqkvA": wqkvA,
            "wqkvB": wqkvB,
            "woutT": woutT,
            "maskT": mT_first if c == 0 else mT_std,
        }
        if use_bqkv:
            im["bqkv"] = b_qkv.reshape(1, 3 * D_MODEL).astype(NP_BF16)
        if use_bout:
            im["bout"] = b_out.reshape(1, D_MODEL).astype(NP_BF16)
        in_maps.append(im)

    res = run_bass_kernel_spmd(nc, in_maps, list(range(N_CORES)))
    LAST_RESULT = res
    out = np.concatenate(
        [res.results[c]["y"].astype(np.float32) for c in range(N_CORES)], axis=0)
    return out.reshape(B, N_SEQ, D_MODEL)



# revision 33
# speedup vs baseline: 1.0149x; 1.0149x over previous
"""Local causal (sliding-window) attention kernel for Trainium2, SPMD over 8 NeuronCores.

Problem: x [1,4096,1024] -> QKV proj -> 16-head attention with causal window 64
         -> out proj. All fp32 at the interface.

Sharding: sequence-parallel. Core c owns queries [512c, 512c+512). Attention is
local (window 64), so each core only needs a 128-row key/value halo (the
previous 128-token block) in addition to its own 512 rows. Each core computes
its full output rows; host concatenates. No collectives.

All on-chip compute is bf16 (fp32 PSUM accumulation): host casts x/weights to
bf16, which halves HBM traffic (the projection phase is DMA-paced) and runs
every matmul at 1 cycle/row regardless of free-dim size. Measured end-to-end
rel err vs the fp32 reference is ~6e-3.

Attention is computed TRANSPOSED (S^T = K^T-stationary x Q): exp(S^T) is
already P^T, so no PE transposes and no P^T staging copies are needed. V tiles
carry an interleaved ones-block per head ([V_h | 1]), so each PV matmul also
accumulates the softmax denominators into psum rows 64:128 for free; the
normalization (x 1/denom) is fused into the psum->sbuf copy of the attention
output (DVE reciprocal + tensor_mul).

Per-core layouts (host pre-transposes so every DMA is a clean row-major tile):
  xT    [1024 d, 640 n]   x^T for rows [s-128, s+512) (core 0: first 128 zero)
  wqkvT [1024 d, 3072 o]  w_qkv^T
  woutT [1024 d, 1024 o]  w_out^T
  maskT [128, 768]        0/1 valid bits for the 5 S^T strips of one head
                          (per-core data; core 0 zeroes the kb0 strip)

S^T strips are trimmed to the query range that can see the strip's keys
(STRIP_Q, 768 columns total); psum banks A=[kb0, kb1, kb4], B=[kb2, kb3];
the sbuf P^T tile is [A | B] = [128, 768]. Strip kb holds keys
k = 128kb + row; entry (row, q) is valid iff q+65 <= k <= q+128.
"""

from contextlib import ExitStack

import ml_dtypes
import numpy as np

import concourse.bass as bass
import concourse.mybir as mybir
import concourse.tile as tile
from concourse import bacc
from concourse.bass_utils import run_bass_kernel_spmd

F32 = mybir.dt.float32
BF16 = mybir.dt.bfloat16
FP8 = mybir.dt.float8e4
NP_BF16 = ml_dtypes.bfloat16
NP_FP8 = ml_dtypes.float8_e4m3
DR = mybir.MatmulPerfMode.DoubleRow


D_MODEL = 1024
N_HEADS = 16
D_HEAD = 64
WINDOW = 64
N_SEQ = 4096
N_CORES = 8
NLOC = N_SEQ // N_CORES          # 512 queries per core
HALO = 128                       # one full key block of halo
NTOT = NLOC + HALO               # 640 local rows (keys/values)
QB = 128                         # query block
NQB = NLOC // QB                 # 4 query blocks per core
KB = 128                         # key block
NKB = NTOT // KB                 # 5 key blocks per core
SCALE = 1.0 / np.sqrt(D_HEAD)

DT = D_MODEL // 128              # 8 contraction tiles

# Diagonal 64-query strips: strip u covers queries [64u, 64u+64) whose full
# key windows [q+65, q+128] all fall inside keys [64(u+1), 64(u+1)+128) --
# exactly 128 keys, so every (head, u) score/PV is ONE matmul with no
# cross-strip accumulation. 8 strips x 64 cols = 512 P^T columns per head
# (vs 768 for 128-key-block strips). Entry (k', q') of strip u is valid iff
# 1 <= k' - q' <= 64 -- the same [128, 64] band mask for every strip.
NSTRIP = 8
STRIP_COLS = 512
# maskT layout: cols 0:512 = the 8 strips' band masks; cols 512:640 = a
# 128x128 identity used by the PE transpose in the V-halo path.
MASKT_COLS = STRIP_COLS + 128

# exposed for test.py (profiling info)
LAST_RESULT = None


def _emit_program(use_bqkv: bool, use_bout: bool, reps: int = 1) -> bass.Bass:
    # Bacc (not raw Bass): its finalize pipeline splits semaphore waits
    # (move_matmul_waits_to_ldweights / generate_event_semaphores) to satisfy
    # the HW limit of 1 sync wait per instruction.
    nc = bacc.Bacc()

    # fp8 hi/lo split operands (see _split8): A = term1 pairs (hi chunk 2j,
    # hi chunk 2j+1), B = cross-term pairs (x: (lo16, hi16); w: (hi16, lo16)).
    xA = nc.declare_dram_parameter("xA", [D_MODEL // 2, 2 * NTOT], FP8, isOutput=False)
    xB = nc.declare_dram_parameter("xB", [D_MODEL, 2 * NTOT], FP8, isOutput=False)
    wqkvA = nc.declare_dram_parameter("wqkvA", [D_MODEL // 2, 3 * 2 * D_MODEL], FP8, isOutput=False)
    wqkvB = nc.declare_dram_parameter("wqkvB", [D_MODEL, 3 * 2 * D_MODEL], FP8, isOutput=False)
    woutT = nc.declare_dram_parameter("woutT", [D_MODEL, D_MODEL], BF16, isOutput=False)
    maskT = nc.declare_dram_parameter("maskT", [128, MASKT_COLS], BF16, isOutput=False)
    if use_bqkv:
        bqkv = nc.declare_dram_parameter("bqkv", [1, 3 * D_MODEL], BF16, isOutput=False)
    if use_bout:
        bout = nc.declare_dram_parameter("bout", [1, D_MODEL], BF16, isOutput=False)
    y = nc.declare_dram_parameter("y", [NLOC, D_MODEL], BF16, isOutput=True)

    with tile.TileContext(nc) as tc:
      for _rep in range(reps):
       with ExitStack() as ctx:
        consts = ctx.enter_context(tc.tile_pool(name="consts", bufs=1))
        xpool = ctx.enter_context(tc.tile_pool(name="xpool", bufs=1))
        wpool = ctx.enter_context(tc.tile_pool(name="wpool", bufs=44))
        qtp = ctx.enter_context(tc.tile_pool(name="qtp", bufs=1))
        ktp = ctx.enter_context(tc.tile_pool(name="ktp", bufs=1))
        vp = ctx.enter_context(tc.tile_pool(name="vp", bufs=1))
        aop = ctx.enter_context(tc.tile_pool(name="aop", bufs=1))
        work = ctx.enter_context(tc.tile_pool(name="work", bufs=2))
        outp = ctx.enter_context(tc.tile_pool(name="outp", bufs=6))
        # PSUM: four role-dedicated tags x 2 slots (1 bank each). Each tag's
        # tiles are read by exactly one engine, keeping matmul wait counts <=2.
        psum = ctx.enter_context(tc.tile_pool(name="psum", bufs=2, space="PSUM"))

        # psum->sbuf copies get EXPLICIT engines: every psum tag must have a
        # single reader engine so a consumer matmul's waits stay within the
        # HW limit of 2 sync-wait commands (producer sem + WAR sem).
        def copy_act(dst, src):
            nc.scalar.copy(dst, src)

        def copy_dve(dst, src):
            nc.vector.tensor_copy(dst, src)

        # During the DMA-paced projection phases the attention psum tags are
        # idle; rotating projection psums across all four tags gives 8 chains
        # in flight instead of 2 (Bacc splits any extra semaphore waits).
        ps_rot = ["ps", "s", "pt", "pA"]
        ps_idx = [0]

        def next_ps(cols, nm):
            tag = ps_rot[ps_idx[0] % 4]
            ps_idx[0] += 1
            return psum.tile([128, cols], F32, tag=tag, name=nm, bufs=2)

        # ---- load x^T (8 tiles [128, 640]) interleaved with wv so the V
        # accumulation chains can start as soon as the first pair lands ----
        # PE pstate ramp: the tensor engine reaches full clock 3us after its
        # first instruction. Fire a trivial matmul on locally-memset data
        # immediately (no DMA dependency) so the ramp clock starts at ~t=0.3us
        # instead of ~2.4us when the first loads land (~1us saved).
        zt = consts.tile([1, 8], BF16, tag="zt")
        nc.gpsimd.memset(zt, 0.0)
        warm_ps = psum.tile([1, 8], F32, tag="pt", name="warm", bufs=2)
        nc.tensor.matmul(warm_ps, lhsT=zt[0:1, 0:1], rhs=zt[0:1, 0:8],
                         start=True, stop=True)

        # All projection chains run fp8 DoubleRow (0.5 cyc/row, 256-wide
        # contraction): out = x_hi w_hi (4 A-pair matmuls) + x_lo w_hi +
        # x_hi w_lo (8 B-pair matmuls) -- bf16-level accuracy at 0.75x the
        # bf16 PE cost. The dropped lo*lo term is ~2^-8 relative.
        xtA = [xpool.tile([128, 2 * NTOT], FP8, tag=f"xA{j}", name=f"xA{j}")
               for j in range(DT // 2)]
        xtB = [xpool.tile([128, 2 * NTOT], FP8, tag=f"xB{g}", name=f"xB{g}")
               for g in range(DT)]
        wvA = [wpool.tile([128, 2 * D_MODEL], FP8, tag="w", name=f"wvA{j}")
               for j in range(DT // 2)]
        wvB = [wpool.tile([128, 2 * D_MODEL], FP8, tag="w", name=f"wvB{g}")
               for g in range(DT)]
        WVC = 2 * 2 * D_MODEL            # wqkvA/B column offset of the V proj
        for j in range(DT // 2):
            nc.sync.dma_start(out=xtA[j], in_=xA[j * 128:(j + 1) * 128, :])
            # wv rides the idle Pool queue so it transfers in parallel with
            # the x stream and the first V matmul can start ~500ns earlier
            nc.gpsimd.dma_start(out=wvA[j], in_=wqkvA[j * 128:(j + 1) * 128, WVC:WVC + 2 * D_MODEL])
        for g in range(DT):
            nc.sync.dma_start(out=xtB[g], in_=xB[g * 128:(g + 1) * 128, :])
            nc.gpsimd.dma_start(out=wvB[g], in_=wqkvB[g * 128:(g + 1) * 128, WVC:WVC + 2 * D_MODEL])

        def pr2(t):
            # pair view [128, 2, C] of an interleaved-pair tile [128, 2C]
            return t[:, :].rearrange("p (two c) -> p two c", two=2)

        def fp8_chain(ps, ocols, xcols, wA, wB, wc0, wcols, w_stationary, stop_last):
            # 12-matmul DoubleRow chain contracting 1024: 4 A-pairs + 8 B-pairs.
            # w_stationary: lhsT = w pairs (Q/K orientation), else lhsT = x pairs.
            for j in range(DT // 2):
                xap = pr2(xtA[j])[:, :, xcols[0]:xcols[1]]
                wap = pr2(wA[j])[:, :, wc0:wc0 + wcols]
                nc.tensor.matmul(
                    ps, lhsT=wap if w_stationary else xap,
                    rhs=xap if w_stationary else wap,
                    start=(j == 0), stop=False, perf_mode=DR)
            for g in range(DT):
                xbp = pr2(xtB[g])[:, :, xcols[0]:xcols[1]]
                wbp = pr2(wB[g])[:, :, wc0:wc0 + wcols]
                nc.tensor.matmul(
                    ps, lhsT=wbp if w_stationary else xbp,
                    rhs=xbp if w_stationary else wbp,
                    start=False, stop=(g == DT - 1 and stop_last), perf_mode=DR)

        # ---- constants, queued AFTER the x/wv stream (not needed until the
        # attention phase; keeping them off the head of the DMA queue lets PE
        # start ~1us earlier) ----
        mT = consts.tile([128, MASKT_COLS], BF16, tag="mT")
        nc.sync.dma_start(out=mT, in_=maskT[:, :])
        if use_bqkv or use_bout:
            ones = consts.tile([1, 512], BF16, tag="ones")
            nc.vector.memset(ones, 1.0)
        if use_bqkv:
            bqkv_sb = consts.tile([1, 3 * D_MODEL], BF16, tag="bqkv")
            nc.sync.dma_start(out=bqkv_sb, in_=bqkv[:, :])
        if use_bout:
            bout_sb = consts.tile([1, D_MODEL], BF16, tag="bout")
            nc.sync.dma_start(out=bout_sb, in_=bout[:, :])

        # ---- Phase V: V'[n, h*128+(0:64)] = (x @ wv^T)_h, V'[n, h*128+(64:128)] = 1
        # The interleaved ones-blocks make every PV matmul accumulate the
        # softmax denominators into psum rows 64:128 at zero PE cost. ----
        vt = [None]
        for n in range(1, NKB):
            t = vp.tile([128, N_HEADS * 128], BF16, tag=f"v{n}", name=f"v{n}")
            onesview = t[:, :].rearrange("p (h c) -> p h c", c=128)[:, :, D_HEAD:128]
            nc.vector.memset(onesview, 1.0)
            vt.append(t)
        for n in range(1, NKB):
            for oh in range(2):
                ps = next_ps(512, "psv")
                fp8_chain(ps, 512, (n * 128, (n + 1) * 128), wvA, wvB,
                          oh * 512, 512, w_stationary=False,
                          stop_last=not use_bqkv)
                if use_bqkv:
                    nc.tensor.matmul(
                        ps, lhsT=ones[0:1, 0:128],
                        rhs=bqkv_sb[0:1, 2 * D_MODEL + oh * 512:2 * D_MODEL + (oh + 1) * 512],
                        start=False, stop=True)
                # strided copy: head j of this half -> V' block (8*oh+j)*128
                dst = vt[n][:, oh * 1024:(oh + 1) * 1024].rearrange(
                    "p (h c) -> p h c", c=128)[:, :, 0:D_HEAD]
                src = ps[:, :].rearrange("p (h c) -> p h c", c=D_HEAD)
                copy_dve(dst, src)
        # Even-u PV strips need V' rows at 64-skewed offsets (64+128j : 192+128j),
        # which straddle two A-tiles; build skewed B-tiles by plain partition-
        # shifted SBUF copies (ones blocks come along for free).
        vtB = []
        for j in range(NKB - 1):
            t = vp.tile([128, N_HEADS * 128], BF16, tag=f"vB{j}", name=f"vB{j}")
            # DVE is free between the V-phase psum copies and the first PV
            # normalize; Pool's queue is needed for the wv/wk DMA streams.
            if j > 0:
                nc.vector.tensor_copy(t[0:64, :], vt[j][64:128, :])
            nc.vector.tensor_copy(t[64:128, :], vt[j + 1][0:64, :])
            vtB.append(t)
        # V-halo (rows 64:128) -> vtB[0] top half, odim-major + PE transpose.
        # Block 0's keys-major chain would cost the full 2x8x512 F for 64 live
        # rows; instead each odim-tile (2 heads) is an 8-matmul F=64 chain into
        # psum [128 odims, 64 keys], staged to sbuf, transposed on PE via the
        # identity in mT, and strided-copied into the two heads' value blocks
        # (5120 cycles instead of 8192).
        onesview0 = vtB[0][0:64, :].rearrange("p (h c) -> p h c", c=128)[:, :, D_HEAD:128]
        nc.vector.memset(onesview0, 1.0)
        for t8 in range(DT):
            hv = psum.tile([128, 64], F32, tag="s", name="hv", bufs=2)
            fp8_chain(hv, 64, (64, 128), wvA, wvB,
                      t8 * 128, 128, w_stationary=True, stop_last=True)
            hv_sb = work.tile([128, 64], BF16, tag="hvs", bufs=2, name="hvs")
            copy_act(hv_sb, hv)
            hv_t = psum.tile([64, 128], BF16, tag="pA", name="hvt", bufs=2)
            nc.tensor.transpose(hv_t, hv_sb, mT[:, STRIP_COLS:STRIP_COLS + 128])
            dst = vtB[0][0:64, t8 * 256:(t8 + 1) * 256].rearrange(
                "p (h c) -> p h c", c=128)[:, :, 0:D_HEAD]
            src = hv_t[:, :].rearrange("p (h c) -> p h c", c=D_HEAD)
            copy_dve(dst, src)

        # ---- Phase Q/K + attention, software-pipelined ----
        # Head pairs are processed in order [1..6 in-loop, then 7, then 0]:
        # the LAST pair processed (0) uses qt/kt tiles ready since o=0, so the
        # tail never waits on fresh projection copies; the out-proj chains
        # contract g=0 last for the same reason.
        wqA, wqB, wkA, wkB = [], [], [], []
        for j in range(DT // 2):
            t = wpool.tile([128, 2 * D_MODEL], FP8, tag="w", name=f"wqA{j}")
            nc.sync.dma_start(out=t, in_=wqkvA[j * 128:(j + 1) * 128, 0:2 * D_MODEL])
            wqA.append(t)
            t = wpool.tile([128, 2 * D_MODEL], FP8, tag="w", name=f"wkA{j}")
            nc.sync.dma_start(out=t, in_=wqkvA[j * 128:(j + 1) * 128, 2 * D_MODEL:4 * D_MODEL])
            wkA.append(t)
        for g in range(DT):
            t = wpool.tile([128, 2 * D_MODEL], FP8, tag="w", name=f"wqB{g}")
            nc.sync.dma_start(out=t, in_=wqkvB[g * 128:(g + 1) * 128, 0:2 * D_MODEL])
            wqB.append(t)
            t = wpool.tile([128, 2 * D_MODEL], FP8, tag="w", name=f"wkB{g}")
            nc.gpsimd.dma_start(out=t, in_=wqkvB[g * 128:(g + 1) * 128, 2 * D_MODEL:4 * D_MODEL])
            wkB.append(t)

        wo = []
        for g in range(DT):
            t = wpool.tile([128, D_MODEL], BF16, tag="w", name=f"wo{g}")
            nc.sync.dma_start(out=t, in_=woutT[g * 128:(g + 1) * 128, :])
            wo.append(t)

        qt = [qtp.tile([128, NLOC], BF16, tag=f"qt{o}", name=f"qt{o}") for o in range(DT)]
        kt = [ktp.tile([128, NTOT], BF16, tag=f"kt{o}", name=f"kt{o}") for o in range(DT)]
        # Keys 0:64 can never be attended (query q sees keys >= q+65) and no
        # diagonal strip reads them (strip u starts at key 64(u+1) >= 64), so
        # kt cols 0:64 are simply never written.
        ao = [aop.tile([128, NLOC], BF16, tag=f"ao{g}", name=f"ao{g}") for g in range(DT)]

        def emit_qk(o, split_copies=False):
            # QT o-tile: out [128 o, 512 n]; rhs = own rows = xT cols [128, 640)
            # Exp and Copy share an ACT function-set table (act_info.json:
            # exp_and_others), so alternating them costs no table reloads
            cp = copy_act

            def copy_out(dst, src):
                if split_copies:
                    # halve the copies so head 2o's scores (rows 0:64) can
                    # issue after the first half lands (shortens the tail)
                    cp(dst[0:64], src[0:64])
                    cp(dst[64:128], src[64:128])
                else:
                    cp(dst, src)

            ps = next_ps(512, "psq")
            fp8_chain(ps, 512, (HALO, NTOT), wqA, wqB,
                      o * 128, 128, w_stationary=True,
                      stop_last=not use_bqkv)
            if use_bqkv:
                nc.tensor.matmul(
                    ps, lhsT=bqkv_sb[0:1, o * 128:(o + 1) * 128],
                    rhs=ones[0:1, 0:512], start=False, stop=True)
            copy_out(qt[o], ps)
            # KT o-tile: rows 64:640 (dead halo cols skipped), two N=288 chains
            for (c0, cw) in ((64, 288), (352, 288)):
                ps = next_ps(cw, "pskt")
                fp8_chain(ps[:, 0:cw], cw, (c0, c0 + cw), wkA, wkB,
                          o * 128, 128, w_stationary=True,
                          stop_last=not use_bqkv)
                if use_bqkv:
                    nc.tensor.matmul(
                        ps[:, 0:cw], lhsT=bqkv_sb[0:1, D_MODEL + o * 128:D_MODEL + (o + 1) * 128],
                        rhs=ones[0:1, 0:cw], start=False, stop=True)
                copy_out(kt[o][:, c0:c0 + cw], ps[:, 0:cw])

        head_state = {}

        def emit_head_scores(h):
            g = h // 2
            r0 = (h % 2) * D_HEAD          # row offset of head h inside tile g
            # S^T diagonal strips into ONE psum bank [128, 512]. The first
            # matmul carries start=True (marks the whole bank pending), later
            # ones first-touch-overwrite their regions, the last carries stop.
            s_ps = psum.tile([128, STRIP_COLS], F32, tag="s", name="sS", bufs=2)
            mm = nc.tensor.matmul
            for u in range(NSTRIP):
                mm(s_ps[:, 64 * u:64 * u + 64],
                   lhsT=kt[g][r0:r0 + D_HEAD, 64 * (u + 1):64 * (u + 1) + 128],
                   rhs=qt[g][r0:r0 + D_HEAD, 64 * u:64 * u + 64],
                   start=(u == 0), stop=(u == NSTRIP - 1), skip_group_check=True)
            # P^T = exp(SCALE * S^T); invalid entries hold finite junk
            # (|SCALE*s| <~ 12, no bf16 overflow), zeroed by the mask below.
            pt_t = work.tile([128, STRIP_COLS], BF16, tag="p", bufs=6, name=f"pt{h}")
            nc.scalar.activation(pt_t[:, 0:256], s_ps[:, 0:256],
                                 mybir.ActivationFunctionType.Exp,
                                 bias=0.0, scale=float(SCALE))
            nc.scalar.activation(pt_t[:, 256:512], s_ps[:, 256:512],
                                 mybir.ActivationFunctionType.Exp,
                                 bias=0.0, scale=float(SCALE))
            # zero the out-of-band entries (Pool; otherwise idle here).
            # Two halves, each pipelined behind its exp, to shorten the
            # exp->mask->PV round trip.
            nc.gpsimd.tensor_mul(pt_t[:, 0:256], pt_t[:, 0:256], mT[:, 0:256])
            nc.gpsimd.tensor_mul(pt_t[:, 256:512], pt_t[:, 256:512], mT[:, 256:512])
            head_state[h] = pt_t

        def emit_head_pv(h):
            g = h // 2
            r0 = (h % 2) * D_HEAD
            pt_t = head_state.pop(h)
            # out'_h [128, 512 q]: rows 0:64 = out_h^T, rows 64:128 = softmax
            # denominators (from the V' ones-blocks). Each strip u is a single
            # matmul over its 128-key window: odd u hits an aligned A-tile,
            # even u the 64-skewed B-tile.
            op = psum.tile([128, NLOC], F32, tag="pA", name="opsum", bufs=2)
            mm = nc.tensor.matmul
            for u in range(NSTRIP):
                vtile = vt[(u + 1) // 2] if u % 2 == 1 else vtB[u // 2]
                mm(op[:, 64 * u:64 * u + 64],
                   lhsT=vtile[:, h * 128:(h + 1) * 128],
                   rhs=pt_t[:, 64 * u:64 * u + 64],
                   start=(u == 0), stop=(u == NSTRIP - 1), skip_group_check=True)
            # normalize fused into the psum->sbuf copy: ao = out * (1/denom)
            # (DVE divide is rejected by the BIR verifier - no divide ALU;
            # Pool cannot read PSUM; ACT-copy decoupling adds a second psum
            # reader engine whose WAR semaphores cost more than it saves)
            rbb = work.tile([D_HEAD, NLOC], F32, tag="rbb", bufs=2, name="rbb")
            nc.vector.reciprocal(rbb, op[D_HEAD:128, :])
            nc.vector.tensor_mul(ao[g][r0:r0 + D_HEAD, :], op[0:D_HEAD, :], rbb)

        for o in range(DT):
            emit_qk(o)
            if o >= 3:
                emit_head_pv(2 * (o - 2))
                emit_head_pv(2 * (o - 2) + 1)
            if o >= 2:
                emit_head_scores(2 * (o - 1))
                emit_head_scores(2 * (o - 1) + 1)
        # tail: pair 0 (ancient tiles) and pair 7; PE filler (pv 12/13, dmy)
        # covers the exp->mask round trips of the last-scored pairs.
        emit_head_scores(0)
        emit_head_scores(1)
        emit_head_scores(14)
        emit_head_scores(15)
        emit_head_pv(12)
        emit_head_pv(13)
        # 8 trivial matmuls make PE observe every wo DMA queue semaphore
        # here (satisfied by now - wo was prefetched), so phase C's matmuls
        # don't each need a DMA wait slot (HW limit: 2 sync waits per matmul)
        dmy = psum.tile([1, 1], F32, tag="pt", name="dmy", bufs=2)
        for g in range(DT):
            nc.tensor.matmul(dmy, lhsT=wo[g][0:1, 0:1],
                             rhs=wo[g][0:1, 0:1],
                             start=(g == 0), stop=(g == DT - 1))
        emit_head_pv(0)
        emit_head_pv(1)
        emit_head_pv(14)
        emit_head_pv(15)

        # ---- Phase C: out = attnout @ wout^T (+ b_out); g=0 contracted last
        # so the chains only need ao[0] (heads 0/1, finishing on DVE) at the
        # very end of each chain ----
        gorder = [1, 2, 3, 4, 5, 6, 7, 0]

        chain_idx = [0]

        def outproj_chain(n, c0, cw, ps=None, cp=None, dma_eng=None):
            if ps is None:
                ps = next_ps(cw, "psc")
            if cp is None:
                cp = copy_dve
            for gi, g in enumerate(gorder):
                nc.tensor.matmul(
                    ps, lhsT=ao[g][:, n * 128:(n + 1) * 128],
                    rhs=wo[g][:, c0:c0 + cw],
                    start=(gi == 0), stop=(gi == DT - 1 and not use_bout))
            if use_bout:
                nc.tensor.matmul(
                    ps, lhsT=ones[0:1, 0:128],
                    rhs=bout_sb[0:1, c0:c0 + cw],
                    start=False, stop=True)
            # y is written in bf16 (host casts back to fp32): halves the DMA
            # bytes and doubles the DVE copy rate on the drain path.
            ot = outp.tile([128, cw], BF16, tag="out", name=f"ot{cw}")
            cp(ot, ps)
            if dma_eng is None:
                # alternate the y DMAs between the SP and Pool queues so the
                # drain's transfers overlap instead of serializing on SP
                dma_eng = nc.sync if chain_idx[0] % 2 == 0 else nc.gpsimd
            chain_idx[0] += 1
            dma_eng.dma_start(out=y[n * 128:(n + 1) * 128, c0:c0 + cw], in_=ot)

        for n in range(NQB):
            for oh in range(2):
                if n == NQB - 1 and oh == 1:
                    # drain choreography: the 384's DVE copy + ACT-queue DMA
                    # and the final 128's ACT copy (psum tag "s" is ACT-read)
                    # + SP DMA run on disjoint engine/queue pairs, so the two
                    # last completions overlap instead of serializing.
                    outproj_chain(n, 512, 384, dma_eng=nc.scalar)
                    fps = psum.tile([128, 128], F32, tag="s", name="fin", bufs=2)
                    outproj_chain(n, 896, 128, ps=fps, cp=copy_act,
                                  dma_eng=nc.sync)
                else:
                    outproj_chain(n, oh * 512, 512)

    return nc


_PROG_CACHE: dict = {}


def _get_program(use_bqkv: bool, use_bout: bool) -> bass.Bass:
    key = (use_bqkv, use_bout)
    if key not in _PROG_CACHE:
        nc = _emit_program(use_bqkv, use_bout)
        if not nc.is_finalized():
            nc.finalize()
        _PROG_CACHE[key] = nc
    return _PROG_CACHE[key]


def _build_maskT(core0: bool) -> np.ndarray:
    """0/1 valid bits for the 8 diagonal S^T strips, [128 k-rows, 512 cols].

    Strip u holds keys 64(u+1)+k' vs queries 64u+q'; valid iff
    1 <= k'-q' <= 64. On core 0 the u=0 strip's keys 64:128 (k' < 64) are
    x-padding, so they are masked off too."""
    m = np.zeros((128, MASKT_COLS), np.float32)
    kp = np.arange(128)[:, None]
    qp = np.arange(64)[None, :]
    band = (kp - qp >= 1) & (kp - qp <= WINDOW)
    for u in range(NSTRIP):
        v = band if not (core0 and u == 0) else band & (kp >= 64)
        m[:, 64 * u:64 * u + 64] = v
    m[:, STRIP_COLS:STRIP_COLS + 128] = np.eye(128, dtype=np.float32)
    return m


def _split8(a):
    """fp8 hi/lo split of an fp32 array: a ~= hi + lo16/16; hi16 = hi/16.

    The 16x scalings keep the lo and scaled-hi values inside e4m3's normal
    range so the cross terms (x_lo w_hi + x_hi w_lo) come out exactly scaled:
    x_lo16 . w_hi16 = x_lo . w_hi, etc."""
    hi = a.astype(NP_FP8)
    hif = hi.astype(np.float32)
    lo16 = ((a - hif) * 16.0).astype(NP_FP8)
    hi16 = (hif / 16.0).astype(NP_FP8)
    return hi, lo16, hi16


def _pairA(hT):
    """[1024, C] -> [512, 2C]: row 128j+p = [chunk(2j) row p | chunk(2j+1) row p]."""
    C = hT.shape[1]
    t = hT.reshape(4, 2, 128, C).transpose(0, 2, 1, 3)
    return np.ascontiguousarray(t.reshape(512, 2 * C))


def _pairB(first, second):
    """two [1024, C] -> [1024, 2C]: row 128g+p = [first(g) row p | second(g) row p]."""
    C = first.shape[1]
    t = np.stack([first.reshape(8, 128, C), second.reshape(8, 128, C)], axis=2)
    return np.ascontiguousarray(t.reshape(1024, 2 * C))


def kernel(x, w_qkv, b_qkv, w_out, b_out):
    global LAST_RESULT
    x = np.asarray(x, dtype=np.float32)
    w_qkv = np.asarray(w_qkv, dtype=np.float32)
    b_qkv = np.asarray(b_qkv, dtype=np.float32)
    w_out = np.asarray(w_out, dtype=np.float32)
    b_out = np.asarray(b_out, dtype=np.float32)

    B = x.shape[0]
    assert x.shape == (1, N_SEQ, D_MODEL), x.shape
    xa = x[0]

    use_bqkv = bool(np.any(b_qkv))
    use_bout = bool(np.any(b_out))
    nc = _get_program(use_bqkv, use_bout)

    wqkvT = np.ascontiguousarray(w_qkv.T)                    # [1024, 3072] f32
    w_hi, w_lo16, w_hi16 = _split8(wqkvT)
    wqkvA = np.concatenate(
        [_pairA(w_hi[:, p * D_MODEL:(p + 1) * D_MODEL]) for p in range(3)], axis=1)
    wqkvB = np.concatenate(
        [_pairB(w_hi16[:, p * D_MODEL:(p + 1) * D_MODEL],
                w_lo16[:, p * D_MODEL:(p + 1) * D_MODEL]) for p in range(3)], axis=1)
    woutT = np.ascontiguousarray(w_out.T).astype(NP_BF16)    # [1024, 1024]
    mT_std = _build_maskT(core0=False).astype(NP_BF16)
    mT_first = _build_maskT(core0=True).astype(NP_BF16)

    in_maps = []
    for c in range(N_CORES):
        s = c * NLOC
        if c == 0:
            blk = np.concatenate([np.zeros((HALO, D_MODEL), np.float32), xa[0:NLOC]], axis=0)
        else:
            blk = xa[s - HALO:s + NLOC]
        xTblk = np.ascontiguousarray(blk.T)                 # [1024, 640] f32
        x_hi, x_lo16, x_hi16 = _split8(xTblk)
        im = {
            "xA": _pairA(x_hi),
            "xB": _pairB(x_lo16, x_hi16),
            "wqkvA": wqkvA,
            "wqkvB": wqkvB,
            "woutT": woutT,
            "maskT": mT_first if c == 0 else mT_std,
        }
        if use_bqkv:
            im["bqkv"] = b_qkv.reshape(1, 3 * D_MODEL).astype(NP_BF16)
        if use_bout:
            im["bout"] = b_out.reshape(1, D_MODEL).astype(NP_BF16)
        in_maps.append(im)

    res = run_bass_kernel_spmd(nc, in_maps, list(range(N_CORES)))
    LAST_RESULT = res
    out = np.concatenate(
        [res.results[c]["y"].astype(np.float32) for c in range(N_CORES)], axis=0)
    return out.reshape(B, N_SEQ, D_MODEL)



# revision 34
# speedup vs baseline: 1.0328x; 1.0176x over previous
"""Local causal (sliding-window) attention kernel for Trainium2, SPMD over 8 NeuronCores.

Problem: x [1,4096,1024] -> QKV proj -> 16-head attention with causal window 64
         -> out proj. All fp32 at the interface.

Sharding: sequence-parallel. Core c owns queries [512c, 512c+512). Attention is
local (window 64), so each core only needs a 128-row key/value halo (the
previous 128-token block) in addition to its own 512 rows. Each core computes
its full output rows; host concatenates. No collectives.

All on-chip compute is bf16 (fp32 PSUM accumulation): host casts x/weights to
bf16, which halves HBM traffic (the projection phase is DMA-paced) and runs
every matmul at 1 cycle/row regardless of free-dim size. Measured end-to-end
rel err vs the fp32 reference is ~6e-3.

Attention is computed TRANSPOSED (S^T = K^T-stationary x Q): exp(S^T) is
already P^T, so no PE transposes and no P^T staging copies are needed. V tiles
carry an interleaved ones-block per head ([V_h | 1]), so each PV matmul also
accumulates the softmax denominators into psum rows 64:128 for free; the
normalization (x 1/denom) is fused into the psum->sbuf copy of the attention
output (DVE reciprocal + tensor_mul).

Per-core layouts (host pre-transposes so every DMA is a clean row-major tile):
  xT    [1024 d, 640 n]   x^T for rows [s-128, s+512) (core 0: first 128 zero)
  wqkvT [1024 d, 3072 o]  w_qkv^T
  woutT [1024 d, 1024 o]  w_out^T
  maskT [128, 768]        0/1 valid bits for the 5 S^T strips of one head
                          (per-core data; core 0 zeroes the kb0 strip)

S^T strips are trimmed to the query range that can see the strip's keys
(STRIP_Q, 768 columns total); psum banks A=[kb0, kb1, kb4], B=[kb2, kb3];
the sbuf P^T tile is [A | B] = [128, 768]. Strip kb holds keys
k = 128kb + row; entry (row, q) is valid iff q+65 <= k <= q+128.
"""

from contextlib import ExitStack

import ml_dtypes
import numpy as np

import concourse.bass as bass
import concourse.mybir as mybir
import concourse.tile as tile
from concourse import bacc
from concourse.bass_utils import run_bass_kernel_spmd

F32 = mybir.dt.float32
BF16 = mybir.dt.bfloat16
FP8 = mybir.dt.float8e4
NP_BF16 = ml_dtypes.bfloat16
NP_FP8 = ml_dtypes.float8_e4m3
DR = mybir.MatmulPerfMode.DoubleRow


D_MODEL = 1024
N_HEADS = 16
D_HEAD = 64
WINDOW = 64
N_SEQ = 4096
N_CORES = 8
NLOC = N_SEQ // N_CORES          # 512 queries per core
HALO = 128                       # one full key block of halo
NTOT = NLOC + HALO               # 640 local rows (keys/values)
QB = 128                         # query block
NQB = NLOC // QB                 # 4 query blocks per core
KB = 128                         # key block
NKB = NTOT // KB                 # 5 key blocks per core
SCALE = 1.0 / np.sqrt(D_HEAD)

DT = D_MODEL // 128              # 8 contraction tiles

# Diagonal 64-query strips: strip u covers queries [64u, 64u+64) whose full
# key windows [q+65, q+128] all fall inside keys [64(u+1), 64(u+1)+128) --
# exactly 128 keys, so every (head, u) score/PV is ONE matmul with no
# cross-strip accumulation. 8 strips x 64 cols = 512 P^T columns per head
# (vs 768 for 128-key-block strips). Entry (k', q') of strip u is valid iff
# 1 <= k' - q' <= 64 -- the same [128, 64] band mask for every strip.
NSTRIP = 8
STRIP_COLS = 512
# maskT layout: cols 0:512 = the 8 strips' band masks; cols 512:640 = a
# 128x128 identity used by the PE transpose in the V-halo path.
MASKT_COLS = STRIP_COLS + 128

# exposed for test.py (profiling info)
LAST_RESULT = None


def _emit_program(use_bqkv: bool, use_bout: bool, reps: int = 1) -> bass.Bass:
    # Bacc (not raw Bass): its finalize pipeline splits semaphore waits
    # (move_matmul_waits_to_ldweights / generate_event_semaphores) to satisfy
    # the HW limit of 1 sync wait per instruction.
    nc = bacc.Bacc()

    # fp8 hi/lo split operands (see _split8): A = term1 pairs (hi chunk 2j,
    # hi chunk 2j+1), B = cross-term pairs (x: (lo16, hi16); w: (hi16, lo16)).
    xA = nc.declare_dram_parameter("xA", [D_MODEL // 2, 2 * NTOT], FP8, isOutput=False)
    xB = nc.declare_dram_parameter("xB", [D_MODEL, 2 * NTOT], FP8, isOutput=False)
    wqkvA = nc.declare_dram_parameter("wqkvA", [D_MODEL // 2, 3 * 2 * D_MODEL], FP8, isOutput=False)
    wqkvB = nc.declare_dram_parameter("wqkvB", [D_MODEL, 3 * 2 * D_MODEL], FP8, isOutput=False)
    woutT = nc.declare_dram_parameter("woutT", [D_MODEL, D_MODEL], BF16, isOutput=False)
    maskT = nc.declare_dram_parameter("maskT", [128, MASKT_COLS], BF16, isOutput=False)
    if use_bqkv:
        bqkv = nc.declare_dram_parameter("bqkv", [1, 3 * D_MODEL], BF16, isOutput=False)
    if use_bout:
        bout = nc.declare_dram_parameter("bout", [1, D_MODEL], BF16, isOutput=False)
    y = nc.declare_dram_parameter("y", [NLOC, D_MODEL], BF16, isOutput=True)

    with tile.TileContext(nc) as tc:
      for _rep in range(reps):
       with ExitStack() as ctx:
        consts = ctx.enter_context(tc.tile_pool(name="consts", bufs=1))
        xpool = ctx.enter_context(tc.tile_pool(name="xpool", bufs=1))
        wpool = ctx.enter_context(tc.tile_pool(name="wpool", bufs=44))
        qtp = ctx.enter_context(tc.tile_pool(name="qtp", bufs=1))
        ktp = ctx.enter_context(tc.tile_pool(name="ktp", bufs=1))
        vp = ctx.enter_context(tc.tile_pool(name="vp", bufs=1))
        aop = ctx.enter_context(tc.tile_pool(name="aop", bufs=1))
        work = ctx.enter_context(tc.tile_pool(name="work", bufs=2))
        outp = ctx.enter_context(tc.tile_pool(name="outp", bufs=6))
        # PSUM: four role-dedicated tags x 2 slots (1 bank each). Each tag's
        # tiles are read by exactly one engine, keeping matmul wait counts <=2.
        psum = ctx.enter_context(tc.tile_pool(name="psum", bufs=2, space="PSUM"))

        # psum->sbuf copies get EXPLICIT engines: every psum tag must have a
        # single reader engine so a consumer matmul's waits stay within the
        # HW limit of 2 sync-wait commands (producer sem + WAR sem).
        def copy_act(dst, src):
            nc.scalar.copy(dst, src)

        def copy_dve(dst, src):
            nc.vector.tensor_copy(dst, src)

        # During the DMA-paced projection phases the attention psum tags are
        # idle; rotating projection psums across all four tags gives 8 chains
        # in flight instead of 2 (Bacc splits any extra semaphore waits).
        ps_rot = ["ps", "s", "pt", "pA"]
        ps_idx = [0]

        def next_ps(cols, nm):
            tag = ps_rot[ps_idx[0] % 4]
            ps_idx[0] += 1
            return psum.tile([128, cols], F32, tag=tag, name=nm, bufs=2)

        # ---- load x^T (8 tiles [128, 640]) interleaved with wv so the V
        # accumulation chains can start as soon as the first pair lands ----
        # PE pstate ramp: the tensor engine reaches full clock 3us after its
        # first instruction. Fire a trivial matmul on locally-memset data
        # immediately (no DMA dependency) so the ramp clock starts at ~t=0.3us
        # instead of ~2.4us when the first loads land (~1us saved).
        zt = consts.tile([1, 8], BF16, tag="zt")
        nc.gpsimd.memset(zt, 0.0)
        warm_ps = psum.tile([1, 8], F32, tag="pt", name="warm", bufs=2)
        nc.tensor.matmul(warm_ps, lhsT=zt[0:1, 0:1], rhs=zt[0:1, 0:8],
                         start=True, stop=True)

        # All projection chains run fp8 DoubleRow (0.5 cyc/row, 256-wide
        # contraction): out = x_hi w_hi (4 A-pair matmuls) + x_lo w_hi +
        # x_hi w_lo (8 B-pair matmuls) -- bf16-level accuracy at 0.75x the
        # bf16 PE cost. The dropped lo*lo term is ~2^-8 relative.
        xtA = [xpool.tile([128, 2 * NTOT], FP8, tag=f"xA{j}", name=f"xA{j}")
               for j in range(DT // 2)]
        xtB = [xpool.tile([128, 2 * NTOT], FP8, tag=f"xB{g}", name=f"xB{g}")
               for g in range(DT)]
        wvA = [wpool.tile([128, 2 * D_MODEL], FP8, tag="w", name=f"wvA{j}")
               for j in range(DT // 2)]
        wvB = [wpool.tile([128, 2 * D_MODEL], FP8, tag="w", name=f"wvB{g}")
               for g in range(DT)]
        WVC = 2 * 2 * D_MODEL            # wqkvA/B column offset of the V proj
        for j in range(DT // 2):
            nc.sync.dma_start(out=xtA[j], in_=xA[j * 128:(j + 1) * 128, :])
            # wv rides the idle Pool queue so it transfers in parallel with
            # the x stream and the first V matmul can start ~500ns earlier
            nc.gpsimd.dma_start(out=wvA[j], in_=wqkvA[j * 128:(j + 1) * 128, WVC:WVC + 2 * D_MODEL])
        for g in range(DT):
            nc.sync.dma_start(out=xtB[g], in_=xB[g * 128:(g + 1) * 128, :])
            nc.gpsimd.dma_start(out=wvB[g], in_=wqkvB[g * 128:(g + 1) * 128, WVC:WVC + 2 * D_MODEL])

        def pr2(t):
            # pair view [128, 2, C] of an interleaved-pair tile [128, 2C]
            return t[:, :].rearrange("p (two c) -> p two c", two=2)

        def fp8_chain(ps, ocols, xcols, wA, wB, wc0, wcols, w_stationary, stop_last):
            # 12-matmul DoubleRow chain contracting 1024: 4 A-pairs + 8 B-pairs.
            # w_stationary: lhsT = w pairs (Q/K orientation), else lhsT = x pairs.
            for j in range(DT // 2):
                xap = pr2(xtA[j])[:, :, xcols[0]:xcols[1]]
                wap = pr2(wA[j])[:, :, wc0:wc0 + wcols]
                nc.tensor.matmul(
                    ps, lhsT=wap if w_stationary else xap,
                    rhs=xap if w_stationary else wap,
                    start=(j == 0), stop=False, perf_mode=DR)
            for g in range(DT):
                xbp = pr2(xtB[g])[:, :, xcols[0]:xcols[1]]
                wbp = pr2(wB[g])[:, :, wc0:wc0 + wcols]
                nc.tensor.matmul(
                    ps, lhsT=wbp if w_stationary else xbp,
                    rhs=xbp if w_stationary else wbp,
                    start=False, stop=(g == DT - 1 and stop_last), perf_mode=DR)

        # ---- constants, queued AFTER the x/wv stream (not needed until the
        # attention phase; keeping them off the head of the DMA queue lets PE
        # start ~1us earlier) ----
        mT = consts.tile([128, MASKT_COLS], BF16, tag="mT")
        nc.sync.dma_start(out=mT, in_=maskT[:, :])
        if use_bqkv or use_bout:
            ones = consts.tile([1, 512], BF16, tag="ones")
            nc.vector.memset(ones, 1.0)
        if use_bqkv:
            bqkv_sb = consts.tile([1, 3 * D_MODEL], BF16, tag="bqkv")
            nc.sync.dma_start(out=bqkv_sb, in_=bqkv[:, :])
        if use_bout:
            bout_sb = consts.tile([1, D_MODEL], BF16, tag="bout")
            nc.sync.dma_start(out=bout_sb, in_=bout[:, :])

        # ---- Phase V: V'[n, h*128+(0:64)] = (x @ wv^T)_h, V'[n, h*128+(64:128)] = 1
        # The interleaved ones-blocks make every PV matmul accumulate the
        # softmax denominators into psum rows 64:128 at zero PE cost. ----
        vt = [None]
        for n in range(1, NKB):
            t = vp.tile([128, N_HEADS * 128], BF16, tag=f"v{n}", name=f"v{n}")
            onesview = t[:, :].rearrange("p (h c) -> p h c", c=128)[:, :, D_HEAD:128]
            nc.vector.memset(onesview, 1.0)
            vt.append(t)
        for n in range(1, NKB):
            for oh in range(2):
                ps = next_ps(512, "psv")
                fp8_chain(ps, 512, (n * 128, (n + 1) * 128), wvA, wvB,
                          oh * 512, 512, w_stationary=False,
                          stop_last=not use_bqkv)
                if use_bqkv:
                    nc.tensor.matmul(
                        ps, lhsT=ones[0:1, 0:128],
                        rhs=bqkv_sb[0:1, 2 * D_MODEL + oh * 512:2 * D_MODEL + (oh + 1) * 512],
                        start=False, stop=True)
                # strided copy: head j of this half -> V' block (8*oh+j)*128
                dst = vt[n][:, oh * 1024:(oh + 1) * 1024].rearrange(
                    "p (h c) -> p h c", c=128)[:, :, 0:D_HEAD]
                src = ps[:, :].rearrange("p (h c) -> p h c", c=D_HEAD)
                copy_dve(dst, src)
        # Even-u PV strips need V' rows at 64-skewed offsets (64+128j : 192+128j),
        # which straddle two A-tiles; build skewed B-tiles by plain partition-
        # shifted SBUF copies (ones blocks come along for free).
        vtB = []
        for j in range(NKB - 1):
            t = vp.tile([128, N_HEADS * 128], BF16, tag=f"vB{j}", name=f"vB{j}")
            # DVE is free between the V-phase psum copies and the first PV
            # normalize; Pool's queue is needed for the wv/wk DMA streams.
            if j > 0:
                nc.vector.tensor_copy(t[0:64, :], vt[j][64:128, :])
            nc.vector.tensor_copy(t[64:128, :], vt[j + 1][0:64, :])
            vtB.append(t)
        # V-halo (rows 64:128) -> vtB[0] top half, odim-major + PE transpose.
        # Block 0's keys-major chain would cost the full 2x8x512 F for 64 live
        # rows; instead each odim-tile (2 heads) is an 8-matmul F=64 chain into
        # psum [128 odims, 64 keys], staged to sbuf, transposed on PE via the
        # identity in mT, and strided-copied into the two heads' value blocks
        # (5120 cycles instead of 8192).
        onesview0 = vtB[0][0:64, :].rearrange("p (h c) -> p h c", c=128)[:, :, D_HEAD:128]
        nc.vector.memset(onesview0, 1.0)
        for t8 in range(DT):
            hv = psum.tile([128, 64], F32, tag="s", name="hv", bufs=2)
            fp8_chain(hv, 64, (64, 128), wvA, wvB,
                      t8 * 128, 128, w_stationary=True, stop_last=True)
            hv_sb = work.tile([128, 64], BF16, tag="hvs", bufs=2, name="hvs")
            copy_act(hv_sb, hv)
            hv_t = psum.tile([64, 128], BF16, tag="pA", name="hvt", bufs=2)
            nc.tensor.transpose(hv_t, hv_sb, mT[:, STRIP_COLS:STRIP_COLS + 128])
            dst = vtB[0][0:64, t8 * 256:(t8 + 1) * 256].rearrange(
                "p (h c) -> p h c", c=128)[:, :, 0:D_HEAD]
            src = hv_t[:, :].rearrange("p (h c) -> p h c", c=D_HEAD)
            copy_dve(dst, src)

        # ---- Phase Q/K + attention, software-pipelined ----
        # Head pairs are processed in order [1..6 in-loop, then 7, then 0]:
        # the LAST pair processed (0) uses qt/kt tiles ready since o=0, so the
        # tail never waits on fresh projection copies; the out-proj chains
        # contract g=0 last for the same reason.
        wqA, wqB, wkA, wkB = [], [], [], []
        for j in range(DT // 2):
            t = wpool.tile([128, 2 * D_MODEL], FP8, tag="w", name=f"wqA{j}")
            nc.sync.dma_start(out=t, in_=wqkvA[j * 128:(j + 1) * 128, 0:2 * D_MODEL])
            wqA.append(t)
            t = wpool.tile([128, 2 * D_MODEL], FP8, tag="w", name=f"wkA{j}")
            nc.gpsimd.dma_start(out=t, in_=wqkvA[j * 128:(j + 1) * 128, 2 * D_MODEL:4 * D_MODEL])
            wkA.append(t)
        for g in range(DT):
            t = wpool.tile([128, 2 * D_MODEL], FP8, tag="w", name=f"wqB{g}")
            nc.sync.dma_start(out=t, in_=wqkvB[g * 128:(g + 1) * 128, 0:2 * D_MODEL])
            wqB.append(t)
            t = wpool.tile([128, 2 * D_MODEL], FP8, tag="w", name=f"wkB{g}")
            nc.gpsimd.dma_start(out=t, in_=wqkvB[g * 128:(g + 1) * 128, 2 * D_MODEL:4 * D_MODEL])
            wkB.append(t)

        wo = []
        for g in range(DT):
            t = wpool.tile([128, D_MODEL], BF16, tag="w", name=f"wo{g}")
            nc.sync.dma_start(out=t, in_=woutT[g * 128:(g + 1) * 128, :])
            wo.append(t)

        qt = [qtp.tile([128, NLOC], BF16, tag=f"qt{o}", name=f"qt{o}") for o in range(DT)]
        kt = [ktp.tile([128, NTOT], BF16, tag=f"kt{o}", name=f"kt{o}") for o in range(DT)]
        # Keys 0:64 can never be attended (query q sees keys >= q+65) and no
        # diagonal strip reads them (strip u starts at key 64(u+1) >= 64), so
        # kt cols 0:64 are simply never written.
        ao = [aop.tile([128, NLOC], BF16, tag=f"ao{g}", name=f"ao{g}") for g in range(DT)]

        def emit_qk(o, split_copies=False):
            # QT o-tile: out [128 o, 512 n]; rhs = own rows = xT cols [128, 640)
            # Exp and Copy share an ACT function-set table (act_info.json:
            # exp_and_others), so alternating them costs no table reloads
            cp = copy_act

            def copy_out(dst, src):
                if split_copies:
                    # halve the copies so head 2o's scores (rows 0:64) can
                    # issue after the first half lands (shortens the tail)
                    cp(dst[0:64], src[0:64])
                    cp(dst[64:128], src[64:128])
                else:
                    cp(dst, src)

            ps = next_ps(512, "psq")
            fp8_chain(ps, 512, (HALO, NTOT), wqA, wqB,
                      o * 128, 128, w_stationary=True,
                      stop_last=not use_bqkv)
            if use_bqkv:
                nc.tensor.matmul(
                    ps, lhsT=bqkv_sb[0:1, o * 128:(o + 1) * 128],
                    rhs=ones[0:1, 0:512], start=False, stop=True)
            copy_out(qt[o], ps)
            # KT o-tile: rows 64:640 (dead halo cols skipped), two N=288 chains
            for (c0, cw) in ((64, 288), (352, 288)):
                ps = next_ps(cw, "pskt")
                fp8_chain(ps[:, 0:cw], cw, (c0, c0 + cw), wkA, wkB,
                          o * 128, 128, w_stationary=True,
                          stop_last=not use_bqkv)
                if use_bqkv:
                    nc.tensor.matmul(
                        ps[:, 0:cw], lhsT=bqkv_sb[0:1, D_MODEL + o * 128:D_MODEL + (o + 1) * 128],
                        rhs=ones[0:1, 0:cw], start=False, stop=True)
                copy_out(kt[o][:, c0:c0 + cw], ps[:, 0:cw])

        head_state = {}

        def emit_head_scores(h):
            g = h // 2
            r0 = (h % 2) * D_HEAD          # row offset of head h inside tile g
            # S^T diagonal strips into ONE psum bank [128, 512]. The first
            # matmul carries start=True (marks the whole bank pending), later
            # ones first-touch-overwrite their regions, the last carries stop.
            s_ps = psum.tile([128, STRIP_COLS], F32, tag="s", name="sS", bufs=2)
            mm = nc.tensor.matmul
            for u in range(NSTRIP):
                mm(s_ps[:, 64 * u:64 * u + 64],
                   lhsT=kt[g][r0:r0 + D_HEAD, 64 * (u + 1):64 * (u + 1) + 128],
                   rhs=qt[g][r0:r0 + D_HEAD, 64 * u:64 * u + 64],
                   start=(u == 0), stop=(u == NSTRIP - 1), skip_group_check=True)
            # P^T = exp(SCALE * S^T); invalid entries hold finite junk
            # (|SCALE*s| <~ 12, no bf16 overflow), zeroed by the mask below.
            pt_t = work.tile([128, STRIP_COLS], BF16, tag="p", bufs=6, name=f"pt{h}")
            nc.scalar.activation(pt_t[:, 0:256], s_ps[:, 0:256],
                                 mybir.ActivationFunctionType.Exp,
                                 bias=0.0, scale=float(SCALE))
            nc.scalar.activation(pt_t[:, 256:512], s_ps[:, 256:512],
                                 mybir.ActivationFunctionType.Exp,
                                 bias=0.0, scale=float(SCALE))
            # zero the out-of-band entries (Pool; otherwise idle here).
            # Two halves, each pipelined behind its exp, to shorten the
            # exp->mask->PV round trip.
            nc.gpsimd.tensor_mul(pt_t[:, 0:256], pt_t[:, 0:256], mT[:, 0:256])
            nc.gpsimd.tensor_mul(pt_t[:, 256:512], pt_t[:, 256:512], mT[:, 256:512])
            head_state[h] = pt_t

        def emit_head_pv(h):
            g = h // 2
            r0 = (h % 2) * D_HEAD
            pt_t = head_state.pop(h)
            # out'_h [128, 512 q]: rows 0:64 = out_h^T, rows 64:128 = softmax
            # denominators (from the V' ones-blocks). Each strip u is a single
            # matmul over its 128-key window: odd u hits an aligned A-tile,
            # even u the 64-skewed B-tile.
            op = psum.tile([128, NLOC], F32, tag="pA", name="opsum", bufs=2)
            mm = nc.tensor.matmul
            for u in range(NSTRIP):
                vtile = vt[(u + 1) // 2] if u % 2 == 1 else vtB[u // 2]
                mm(op[:, 64 * u:64 * u + 64],
                   lhsT=vtile[:, h * 128:(h + 1) * 128],
                   rhs=pt_t[:, 64 * u:64 * u + 64],
                   start=(u == 0), stop=(u == NSTRIP - 1), skip_group_check=True)
            # normalize fused into the psum->sbuf copy: ao = out * (1/denom)
            # (DVE divide is rejected by the BIR verifier - no divide ALU;
            # Pool cannot read PSUM; ACT-copy decoupling adds a second psum
            # reader engine whose WAR semaphores cost more than it saves)
            rbb = work.tile([D_HEAD, NLOC], F32, tag="rbb", bufs=2, name="rbb")
            nc.vector.reciprocal(rbb, op[D_HEAD:128, :])
            nc.vector.tensor_mul(ao[g][r0:r0 + D_HEAD, :], op[0:D_HEAD, :], rbb)

        for o in range(DT):
            emit_qk(o)
            if o >= 3:
                emit_head_pv(2 * (o - 2))
                emit_head_pv(2 * (o - 2) + 1)
            if o >= 2:
                emit_head_scores(2 * (o - 1))
                emit_head_scores(2 * (o - 1) + 1)
        # tail: pair 0 (ancient tiles) and pair 7; PE filler (pv 12/13, dmy)
        # covers the exp->mask round trips of the last-scored pairs.
        emit_head_scores(0)
        emit_head_scores(1)
        emit_head_scores(14)
        emit_head_scores(15)
        emit_head_pv(12)
        emit_head_pv(13)
        # 8 trivial matmuls make PE observe every wo DMA queue semaphore
        # here (satisfied by now - wo was prefetched), so phase C's matmuls
        # don't each need a DMA wait slot (HW limit: 2 sync waits per matmul)
        dmy = psum.tile([1, 1], F32, tag="pt", name="dmy", bufs=2)
        for g in range(DT):
            nc.tensor.matmul(dmy, lhsT=wo[g][0:1, 0:1],
                             rhs=wo[g][0:1, 0:1],
                             start=(g == 0), stop=(g == DT - 1))
        emit_head_pv(0)
        emit_head_pv(1)
        emit_head_pv(14)
        emit_head_pv(15)

        # ---- Phase C: out = attnout @ wout^T (+ b_out); g=0 contracted last
        # so the chains only need ao[0] (heads 0/1, finishing on DVE) at the
        # very end of each chain ----
        gorder = [1, 2, 3, 4, 5, 6, 7, 0]

        chain_idx = [0]

        def outproj_chain(n, c0, cw, ps=None, cp=None, dma_eng=None):
            if ps is None:
                ps = next_ps(cw, "psc")
            if cp is None:
                cp = copy_dve
            for gi, g in enumerate(gorder):
                nc.tensor.matmul(
                    ps, lhsT=ao[g][:, n * 128:(n + 1) * 128],
                    rhs=wo[g][:, c0:c0 + cw],
                    start=(gi == 0), stop=(gi == DT - 1 and not use_bout))
            if use_bout:
                nc.tensor.matmul(
                    ps, lhsT=ones[0:1, 0:128],
                    rhs=bout_sb[0:1, c0:c0 + cw],
                    start=False, stop=True)
            # y is written in bf16 (host casts back to fp32): halves the DMA
            # bytes and doubles the DVE copy rate on the drain path.
            ot = outp.tile([128, cw], BF16, tag="out", name=f"ot{cw}")
            cp(ot, ps)
            if dma_eng is None:
                # alternate the y DMAs between the SP and Pool queues so the
                # drain's transfers overlap instead of serializing on SP
                dma_eng = nc.sync if chain_idx[0] % 2 == 0 else nc.gpsimd
            chain_idx[0] += 1
            dma_eng.dma_start(out=y[n * 128:(n + 1) * 128, c0:c0 + cw], in_=ot)

        for n in range(NQB):
            for oh in range(2):
                if n == NQB - 1 and oh == 1:
                    # drain choreography: the 384's DVE copy + ACT-queue DMA
                    # and the final 128's ACT copy (psum tag "s" is ACT-read)
                    # + SP DMA run on disjoint engine/queue pairs, so the two
                    # last completions overlap instead of serializing.
                    outproj_chain(n, 512, 384, dma_eng=nc.scalar)
                    fps = psum.tile([128, 128], F32, tag="s", name="fin", bufs=2)
                    outproj_chain(n, 896, 128, ps=fps, cp=copy_act,
                                  dma_eng=nc.sync)
                else:
                    outproj_chain(n, oh * 512, 512)

    return nc


_PROG_CACHE: dict = {}


def _get_program(use_bqkv: bool, use_bout: bool) -> bass.Bass:
    key = (use_bqkv, use_bout)
    if key not in _PROG_CACHE:
        nc = _emit_program(use_bqkv, use_bout)
        if not nc.is_finalized():
            nc.finalize()
        _PROG_CACHE[key] = nc
    return _PROG_CACHE[key]


def _build_maskT(core0: bool) -> np.ndarray:
    """0/1 valid bits for the 8 diagonal S^T strips, [128 k-rows, 512 cols].

    Strip u holds keys 64(u+1)+k' vs queries 64u+q'; valid iff
    1 <= k'-q' <= 64. On core 0 the u=0 strip's keys 64:128 (k' < 64) are
    x-padding, so they are masked off too."""
    m = np.zeros((128, MASKT_COLS), np.float32)
    kp = np.arange(128)[:, None]
    qp = np.arange(64)[None, :]
    band = (kp - qp >= 1) & (kp - qp <= WINDOW)
    for u in range(NSTRIP):
        v = band if not (core0 and u == 0) else band & (kp >= 64)
        m[:, 64 * u:64 * u + 64] = v
    m[:, STRIP_COLS:STRIP_COLS + 128] = np.eye(128, dtype=np.float32)
    return m


def _split8(a):
    """fp8 hi/lo split of an fp32 array: a ~= hi + lo16/16; hi16 = hi/16.

    The 16x scalings keep the lo and scaled-hi values inside e4m3's normal
    range so the cross terms (x_lo w_hi + x_hi w_lo) come out exactly scaled:
    x_lo16 . w_hi16 = x_lo . w_hi, etc."""
    hi = a.astype(NP_FP8)
    hif = hi.astype(np.float32)
    lo16 = ((a - hif) * 16.0).astype(NP_FP8)
    hi16 = (hif / 16.0).astype(NP_FP8)
    return hi, lo16, hi16


def _pairA(hT):
    """[1024, C] -> [512, 2C]: row 128j+p = [chunk(2j) row p | chunk(2j+1) row p]."""
    C = hT.shape[1]
    t = hT.reshape(4, 2, 128, C).transpose(0, 2, 1, 3)
    return np.ascontiguousarray(t.reshape(512, 2 * C))


def _pairB(first, second):
    """two [1024, C] -> [1024, 2C]: row 128g+p = [first(g) row p | second(g) row p]."""
    C = first.shape[1]
    t = np.stack([first.reshape(8, 128, C), second.reshape(8, 128, C)], axis=2)
    return np.ascontiguousarray(t.reshape(1024, 2 * C))


def kernel(x, w_qkv, b_qkv, w_out, b_out):
    global LAST_RESULT
    x = np.asarray(x, dtype=np.float32)
    w_qkv = np.asarray(w_qkv, dtype=np.float32)
    b_qkv = np.asarray(b_qkv, dtype=np.float32)
    w_out = np.asarray(w_out, dtype=np.float32)
    b_out = np.asarray(b_out, dtype=np.float32)

    B = x.shape[0]
    assert x.shape == (1, N_SEQ, D_MODEL), x.shape
    xa = x[0]

    use_bqkv = bool(np.any(b_qkv))
    use_bout = bool(np.any(b_out))
    nc = _get_program(use_bqkv, use_bout)

    wqkvT = np.ascontiguousarray(w_qkv.T)                    # [1024, 3072] f32
    w_hi, w_lo16, w_hi16 = _split8(wqkvT)
    wqkvA = np.concatenate(
        [_pairA(w_hi[:, p * D_MODEL:(p + 1) * D_MODEL]) for p in range(3)], axis=1)
    wqkvB = np.concatenate(
        [_pairB(w_hi16[:, p * D_MODEL:(p + 1) * D_MODEL],
                w_lo16[:, p * D_MODEL:(p + 1) * D_MODEL]) for p in range(3)], axis=1)
    woutT = np.ascontiguousarray(w_out.T).astype(NP_BF16)    # [1024, 1024]
    mT_std = _build_maskT(core0=False).astype(NP_BF16)
    mT_first = _build_maskT(core0=True).astype(NP_BF16)

    in_maps = []
    for c in range(N_CORES):
        s = c * NLOC
        if c == 0:
            blk = np.concatenate([np.zeros((HALO, D_MODEL), np.float32), xa[0:NLOC]], axis=0)
        else:
            blk = xa[s - HALO:s + NLOC]
        xTblk = np.ascontiguousarray(blk.T)                 # [1024, 640] f32
        x_hi, x_lo16, x_hi16 = _split8(xTblk)
        im = {
            "xA": _pairA(x_hi),
            "xB": _pairB(x_lo16, x_hi16),
            "wqkvA": wqkvA,
            "wqkvB": wqkvB,
            "woutT": woutT,
            "maskT": mT_first if c == 0 else mT_std,
        }
        if use_bqkv:
            im["bqkv"] = b_qkv.reshape(1, 3 * D_MODEL).astype(NP_BF16)
        if use_bout:
            im["bout"] = b_out.reshape(1, D_MODEL).astype(NP_BF16)
        in_maps.append(im)

    res = run_bass_kernel_spmd(nc, in_maps, list(range(N_CORES)))
    LAST_RESULT = res
    out = np.concatenate(
        [res.results[c]["y"].astype(np.float32) for c in range(N_CORES)], axis=0)
    return out.reshape(B, N_SEQ, D_MODEL)



# revision 37
# speedup vs baseline: 1.0819x; 1.0476x over previous
"""Local causal (sliding-window) attention kernel for Trainium2, SPMD over 8 NeuronCores.

Problem: x [1,4096,1024] -> QKV proj -> 16-head attention with causal window 64
         -> out proj. All fp32 at the interface.

Sharding: sequence-parallel. Core c owns queries [512c, 512c+512). Attention is
local (window 64), so each core only needs a 128-row key/value halo (the
previous 128-token block) in addition to its own 512 rows. Each core computes
its full output rows; host concatenates. No collectives.

All on-chip compute is bf16 (fp32 PSUM accumulation): host casts x/weights to
bf16, which halves HBM traffic (the projection phase is DMA-paced) and runs
every matmul at 1 cycle/row regardless of free-dim size. Measured end-to-end
rel err vs the fp32 reference is ~6e-3.

Attention is computed TRANSPOSED (S^T = K^T-stationary x Q): exp(S^T) is
already P^T, so no PE transposes and no P^T staging copies are needed. V tiles
carry an interleaved ones-block per head ([V_h | 1]), so each PV matmul also
accumulates the softmax denominators into psum rows 64:128 for free; the
normalization (x 1/denom) is fused into the psum->sbuf copy of the attention
output (DVE reciprocal + tensor_mul).

Per-core layouts (host pre-transposes so every DMA is a clean row-major tile):
  xT    [1024 d, 640 n]   x^T for rows [s-128, s+512) (core 0: first 128 zero)
  wqkvT [1024 d, 3072 o]  w_qkv^T
  woutT [1024 d, 1024 o]  w_out^T
  maskT [128, 768]        0/1 valid bits for the 5 S^T strips of one head
                          (per-core data; core 0 zeroes the kb0 strip)

S^T strips are trimmed to the query range that can see the strip's keys
(STRIP_Q, 768 columns total); psum banks A=[kb0, kb1, kb4], B=[kb2, kb3];
the sbuf P^T tile is [A | B] = [128, 768]. Strip kb holds keys
k = 128kb + row; entry (row, q) is valid iff q+65 <= k <= q+128.
"""

from contextlib import ExitStack

import ml_dtypes
import numpy as np

import concourse.bass as bass
import concourse.mybir as mybir
import concourse.tile as tile
from concourse import bacc
from concourse.bass_utils import run_bass_kernel_spmd

F32 = mybir.dt.float32
BF16 = mybir.dt.bfloat16
FP8 = mybir.dt.float8e4
NP_BF16 = ml_dtypes.bfloat16
NP_FP8 = ml_dtypes.float8_e4m3
DR = mybir.MatmulPerfMode.DoubleRow


D_MODEL = 1024
N_HEADS = 16
D_HEAD = 64
WINDOW = 64
N_SEQ = 4096
N_CORES = 8
NLOC = N_SEQ // N_CORES          # 512 queries per core
HALO = 128                       # one full key block of halo
NTOT = NLOC + HALO               # 640 local rows (keys/values)
QB = 128                         # query block
NQB = NLOC // QB                 # 4 query blocks per core
KB = 128                         # key block
NKB = NTOT // KB                 # 5 key blocks per core
SCALE = 1.0 / np.sqrt(D_HEAD)

DT = D_MODEL // 128              # 8 contraction tiles

# Diagonal 64-query strips: strip u covers queries [64u, 64u+64) whose full
# key windows [q+65, q+128] all fall inside keys [64(u+1), 64(u+1)+128) --
# exactly 128 keys, so every (head, u) score/PV is ONE matmul with no
# cross-strip accumulation. 8 strips x 64 cols = 512 P^T columns per head
# (vs 768 for 128-key-block strips). Entry (k', q') of strip u is valid iff
# 1 <= k' - q' <= 64 -- the same [128, 64] band mask for every strip.
NSTRIP = 8
STRIP_COLS = 512
# maskT layout: cols 0:512 = the 8 strips' band masks; cols 512:640 = a
# 128x128 identity used by the PE transpose in the V-halo path.
MASKT_COLS = STRIP_COLS + 128

# exposed for test.py (profiling info)
LAST_RESULT = None

# DMA queue assignment per stream group ("sp" or "pool"), tuned empirically
# (the Tile scheduler is sensitive to both queue choice and emission order).
QMAP = {"xB": "sp", "wqA": "sp", "wqB": "sp", "wkA": "pool", "wkB": "pool",
        "wo": "sp"}


def _emit_program(use_bqkv: bool, use_bout: bool, reps: int = 1) -> bass.Bass:
    # Bacc (not raw Bass): its finalize pipeline splits semaphore waits
    # (move_matmul_waits_to_ldweights / generate_event_semaphores) to satisfy
    # the HW limit of 1 sync wait per instruction.
    nc = bacc.Bacc()
    _q = {}

    # fp8 hi/lo split operands, all in across-g pair layout (_pairA): row
    # 128j+p = [chunk 2j row p | chunk 2j+1 row p]. The host pre-scales wqkv
    # by 32 so the lo residuals of BOTH operands sit in e4m3's normal range
    # unscaled; every product (hi.hi + lo.hi + hi.lo) is then a plain
    # DoubleRow pair-chain over the same four j-tiles per operand.
    xA = nc.declare_dram_parameter("xA", [D_MODEL // 2, 2 * NTOT], FP8, isOutput=False)
    xL = nc.declare_dram_parameter("xL", [D_MODEL // 2, 2 * NTOT], FP8, isOutput=False)
    wqkvA = nc.declare_dram_parameter("wqkvA", [D_MODEL // 2, 3 * 2 * D_MODEL], FP8, isOutput=False)
    wqkvL = nc.declare_dram_parameter("wqkvL", [D_MODEL // 2, 3 * 2 * D_MODEL], FP8, isOutput=False)
    woutT = nc.declare_dram_parameter("woutT", [D_MODEL, D_MODEL], BF16, isOutput=False)
    maskT = nc.declare_dram_parameter("maskT", [128, MASKT_COLS], BF16, isOutput=False)
    if use_bqkv:
        bqkv = nc.declare_dram_parameter("bqkv", [1, 3 * D_MODEL], BF16, isOutput=False)
    if use_bout:
        bout = nc.declare_dram_parameter("bout", [1, D_MODEL], BF16, isOutput=False)
    y = nc.declare_dram_parameter("y", [NLOC, D_MODEL], BF16, isOutput=True)

    def qeng(group):
        return {"sp": nc.sync, "pool": nc.gpsimd}[QMAP[group]]

    with tile.TileContext(nc) as tc:
      for _rep in range(reps):
       with ExitStack() as ctx:
        consts = ctx.enter_context(tc.tile_pool(name="consts", bufs=1))
        xpool = ctx.enter_context(tc.tile_pool(name="xpool", bufs=1))
        wpool = ctx.enter_context(tc.tile_pool(name="wpool", bufs=44))
        qtp = ctx.enter_context(tc.tile_pool(name="qtp", bufs=1))
        ktp = ctx.enter_context(tc.tile_pool(name="ktp", bufs=1))
        vp = ctx.enter_context(tc.tile_pool(name="vp", bufs=1))
        aop = ctx.enter_context(tc.tile_pool(name="aop", bufs=1))
        work = ctx.enter_context(tc.tile_pool(name="work", bufs=2))
        outp = ctx.enter_context(tc.tile_pool(name="outp", bufs=6))
        # PSUM: four role-dedicated tags x 2 slots (1 bank each). Each tag's
        # tiles are read by exactly one engine, keeping matmul wait counts <=2.
        psum = ctx.enter_context(tc.tile_pool(name="psum", bufs=2, space="PSUM"))

        # psum->sbuf copies get EXPLICIT engines: every psum tag must have a
        # single reader engine so a consumer matmul's waits stay within the
        # HW limit of 2 sync-wait commands (producer sem + WAR sem).
        def copy_act(dst, src):
            nc.scalar.copy(dst, src)

        def copy_dve(dst, src):
            nc.vector.tensor_copy(dst, src)

        # During the DMA-paced projection phases the attention psum tags are
        # idle; rotating projection psums across all four tags gives 8 chains
        # in flight instead of 2 (Bacc splits any extra semaphore waits).
        ps_rot = ["ps", "s", "pt", "pA"]
        ps_idx = [0]

        def next_ps(cols, nm):
            tag = ps_rot[ps_idx[0] % 4]
            ps_idx[0] += 1
            return psum.tile([128, cols], F32, tag=tag, name=nm, bufs=2)

        # ---- load x^T (8 tiles [128, 640]) interleaved with wv so the V
        # accumulation chains can start as soon as the first pair lands ----
        # PE pstate ramp: the tensor engine reaches full clock 3us after its
        # first instruction. Fire a trivial matmul on locally-memset data
        # immediately (no DMA dependency) so the ramp clock starts at ~t=0.3us
        # instead of ~2.4us when the first loads land (~1us saved).
        zt = consts.tile([1, 8], BF16, tag="zt")
        nc.gpsimd.memset(zt, 0.0)
        warm_ps = psum.tile([1, 8], F32, tag="pt", name="warm", bufs=2)
        nc.tensor.matmul(warm_ps, lhsT=zt[0:1, 0:1], rhs=zt[0:1, 0:8],
                         start=True, stop=True)

        # All projection chains run fp8 DoubleRow (0.5 cyc/row, 256-wide
        # contraction): out = x_hi w_hi + x_lo w_hi + x_hi w_lo, three
        # across-g pair-chains of 4 matmuls each -- bf16-level accuracy at
        # 0.75x the bf16 PE cost. The dropped lo*lo term is ~2^-8 relative.
        xtA = [xpool.tile([128, 2 * NTOT], FP8, tag=f"xA{j}", name=f"xA{j}")
               for j in range(DT // 2)]
        xtL = [xpool.tile([128, 2 * NTOT], FP8, tag=f"xL{j}", name=f"xL{j}")
               for j in range(DT // 2)]
        wvA = [wpool.tile([128, 2 * D_MODEL], FP8, tag="w", name=f"wvA{j}")
               for j in range(DT // 2)]
        wvL = [wpool.tile([128, 2 * D_MODEL], FP8, tag="w", name=f"wvL{j}")
               for j in range(DT // 2)]
        WVC = 2 * 2 * D_MODEL            # wqkvA/L column offset of the V proj
        for j in range(DT // 2):
            nc.sync.dma_start(out=xtA[j], in_=xA[j * 128:(j + 1) * 128, :])
            # wv rides the idle Pool queue so it transfers in parallel with
            # the x stream and the first V matmul can start ~500ns earlier
            nc.gpsimd.dma_start(out=wvA[j], in_=wqkvA[j * 128:(j + 1) * 128, WVC:WVC + 2 * D_MODEL])
        for j in range(DT // 2):
            qeng("xB").dma_start(out=xtL[j], in_=xL[j * 128:(j + 1) * 128, :])
            nc.gpsimd.dma_start(out=wvL[j], in_=wqkvL[j * 128:(j + 1) * 128, WVC:WVC + 2 * D_MODEL])

        def pr2(t):
            # pair view [128, 2, C] of an interleaved-pair tile [128, 2C]
            return t[:, :].rearrange("p (two c) -> p two c", two=2)

        def fp8_chain(ps, ocols, xcols, wA, wL, wc0, wcols, w_stationary, stop_last):
            # 12-matmul DoubleRow chain contracting 1024: three 4-pair terms.
            # w_stationary: lhsT = w pairs (Q/K orientation), else lhsT = x pairs.
            terms = [(xtA, wA), (xtL, wA), (xtA, wL)]
            for ti, (xs, ws) in enumerate(terms):
                for j in range(DT // 2):
                    xap = pr2(xs[j])[:, :, xcols[0]:xcols[1]]
                    wap = pr2(ws[j])[:, :, wc0:wc0 + wcols]
                    nc.tensor.matmul(
                        ps, lhsT=wap if w_stationary else xap,
                        rhs=xap if w_stationary else wap,
                        start=(ti == 0 and j == 0),
                        stop=(ti == 2 and j == DT // 2 - 1 and stop_last),
                        perf_mode=DR)

        # ---- constants, queued AFTER the x/wv stream (not needed until the
        # attention phase; keeping them off the head of the DMA queue lets PE
        # start ~1us earlier) ----
        mT = consts.tile([128, MASKT_COLS], BF16, tag="mT")
        nc.sync.dma_start(out=mT, in_=maskT[:, :])
        if use_bqkv or use_bout:
            ones = consts.tile([1, 512], BF16, tag="ones")
            nc.vector.memset(ones, 1.0)
        if use_bqkv:
            bqkv_sb = consts.tile([1, 3 * D_MODEL], BF16, tag="bqkv")
            nc.sync.dma_start(out=bqkv_sb, in_=bqkv[:, :])
        if use_bout:
            bout_sb = consts.tile([1, D_MODEL], BF16, tag="bout")
            nc.sync.dma_start(out=bout_sb, in_=bout[:, :])

        # ---- Phase V: V'[n, h*128+(0:64)] = (x @ wv^T)_h, V'[n, h*128+(64:128)] = 1
        # The interleaved ones-blocks make every PV matmul accumulate the
        # softmax denominators into psum rows 64:128 at zero PE cost. ----
        vt = [None]
        for n in range(1, NKB):
            t = vp.tile([128, N_HEADS * 128], BF16, tag=f"v{n}", name=f"v{n}")
            onesview = t[:, :].rearrange("p (h c) -> p h c", c=128)[:, :, D_HEAD:128]
            nc.vector.memset(onesview, 1.0)
            vt.append(t)
        for n in range(1, NKB):
            for oh in range(2):
                ps = next_ps(512, "psv")
                fp8_chain(ps, 512, (n * 128, (n + 1) * 128), wvA, wvL,
                          oh * 512, 512, w_stationary=False,
                          stop_last=not use_bqkv)
                if use_bqkv:
                    nc.tensor.matmul(
                        ps, lhsT=ones[0:1, 0:128],
                        rhs=bqkv_sb[0:1, 2 * D_MODEL + oh * 512:2 * D_MODEL + (oh + 1) * 512],
                        start=False, stop=True)
                # strided copy: head j of this half -> V' block (8*oh+j)*128
                dst = vt[n][:, oh * 1024:(oh + 1) * 1024].rearrange(
                    "p (h c) -> p h c", c=128)[:, :, 0:D_HEAD]
                src = ps[:, :].rearrange("p (h c) -> p h c", c=D_HEAD)
                copy_dve(dst, src)
        # Even-u PV strips need V' rows at 64-skewed offsets (64+128j : 192+128j),
        # which straddle two A-tiles; build skewed B-tiles by plain partition-
        # shifted SBUF copies (ones blocks come along for free).
        vtB = []
        for j in range(NKB - 1):
            t = vp.tile([128, N_HEADS * 128], BF16, tag=f"vB{j}", name=f"vB{j}")
            # DVE is free between the V-phase psum copies and the first PV
            # normalize; Pool's queue is needed for the wv/wk DMA streams.
            if j > 0:
                nc.vector.tensor_copy(t[0:64, :], vt[j][64:128, :])
            nc.vector.tensor_copy(t[64:128, :], vt[j + 1][0:64, :])
            vtB.append(t)
        # V-halo (rows 64:128) -> vtB[0] top half, odim-major + PE transpose.
        # Block 0's keys-major chain would cost the full 2x8x512 F for 64 live
        # rows; instead each odim-tile (2 heads) is an 8-matmul F=64 chain into
        # psum [128 odims, 64 keys], staged to sbuf, transposed on PE via the
        # identity in mT, and strided-copied into the two heads' value blocks
        # (5120 cycles instead of 8192).
        onesview0 = vtB[0][0:64, :].rearrange("p (h c) -> p h c", c=128)[:, :, D_HEAD:128]
        nc.vector.memset(onesview0, 1.0)
        for t8 in range(DT):
            hv = psum.tile([128, 64], F32, tag="s", name="hv", bufs=2)
            fp8_chain(hv, 64, (64, 128), wvA, wvL,
                      t8 * 128, 128, w_stationary=True, stop_last=True)
            hv_sb = work.tile([128, 64], BF16, tag="hvs", bufs=2, name="hvs")
            copy_act(hv_sb, hv)
            hv_t = psum.tile([64, 128], BF16, tag="pA", name="hvt", bufs=2)
            nc.tensor.transpose(hv_t, hv_sb, mT[:, STRIP_COLS:STRIP_COLS + 128])
            dst = vtB[0][0:64, t8 * 256:(t8 + 1) * 256].rearrange(
                "p (h c) -> p h c", c=128)[:, :, 0:D_HEAD]
            src = hv_t[:, :].rearrange("p (h c) -> p h c", c=D_HEAD)
            copy_dve(dst, src)

        # ---- Phase Q/K + attention, software-pipelined ----
        # Head pairs are processed in order [1..6 in-loop, then 7, then 0]:
        # the LAST pair processed (0) uses qt/kt tiles ready since o=0, so the
        # tail never waits on fresh projection copies; the out-proj chains
        # contract g=0 last for the same reason.
        wqA, wqL, wkA, wkL = [], [], [], []
        for j in range(DT // 2):
            t = wpool.tile([128, 2 * D_MODEL], FP8, tag="w", name=f"wqA{j}")
            qeng("wqA").dma_start(out=t, in_=wqkvA[j * 128:(j + 1) * 128, 0:2 * D_MODEL])
            wqA.append(t)
            t = wpool.tile([128, 2 * D_MODEL], FP8, tag="w", name=f"wkA{j}")
            qeng("wkA").dma_start(out=t, in_=wqkvA[j * 128:(j + 1) * 128, 2 * D_MODEL:4 * D_MODEL])
            wkA.append(t)
        for j in range(DT // 2):
            t = wpool.tile([128, 2 * D_MODEL], FP8, tag="w", name=f"wqL{j}")
            qeng("wqB").dma_start(out=t, in_=wqkvL[j * 128:(j + 1) * 128, 0:2 * D_MODEL])
            wqL.append(t)
            t = wpool.tile([128, 2 * D_MODEL], FP8, tag="w", name=f"wkL{j}")
            qeng("wkB").dma_start(out=t, in_=wqkvL[j * 128:(j + 1) * 128, 2 * D_MODEL:4 * D_MODEL])
            wkL.append(t)

        wo = []
        for g in range(DT):
            t = wpool.tile([128, D_MODEL], BF16, tag="w", name=f"wo{g}")
            qeng("wo").dma_start(out=t, in_=woutT[g * 128:(g + 1) * 128, :])
            wo.append(t)

        qt = [qtp.tile([128, NLOC], BF16, tag=f"qt{o}", name=f"qt{o}") for o in range(DT)]
        kt = [ktp.tile([128, NTOT], BF16, tag=f"kt{o}", name=f"kt{o}") for o in range(DT)]
        # Keys 0:64 can never be attended (query q sees keys >= q+65) and no
        # diagonal strip reads them (strip u starts at key 64(u+1) >= 64), so
        # kt cols 0:64 are simply never written.
        ao = [aop.tile([128, NLOC], BF16, tag=f"ao{g}", name=f"ao{g}") for g in range(DT)]

        def emit_qk(o, split_copies=False):
            # QT o-tile: out [128 o, 512 n]; rhs = own rows = xT cols [128, 640)
            # Exp and Copy share an ACT function-set table (act_info.json:
            # exp_and_others), so alternating them costs no table reloads
            cp = copy_act

            def copy_out(dst, src):
                if split_copies:
                    # halve the copies so head 2o's scores (rows 0:64) can
                    # issue after the first half lands (shortens the tail)
                    cp(dst[0:64], src[0:64])
                    cp(dst[64:128], src[64:128])
                else:
                    cp(dst, src)

            ps = next_ps(512, "psq")
            fp8_chain(ps, 512, (HALO, NTOT), wqA, wqL,
                      o * 128, 128, w_stationary=True,
                      stop_last=not use_bqkv)
            if use_bqkv:
                nc.tensor.matmul(
                    ps, lhsT=bqkv_sb[0:1, o * 128:(o + 1) * 128],
                    rhs=ones[0:1, 0:512], start=False, stop=True)
            copy_out(qt[o], ps)
            # KT o-tile: rows 64:640 (dead halo cols skipped), two N=288 chains
            for (c0, cw) in ((64, 288), (352, 288)):
                ps = next_ps(cw, "pskt")
                fp8_chain(ps[:, 0:cw], cw, (c0, c0 + cw), wkA, wkL,
                          o * 128, 128, w_stationary=True,
                          stop_last=not use_bqkv)
                if use_bqkv:
                    nc.tensor.matmul(
                        ps[:, 0:cw], lhsT=bqkv_sb[0:1, D_MODEL + o * 128:D_MODEL + (o + 1) * 128],
                        rhs=ones[0:1, 0:cw], start=False, stop=True)
                copy_out(kt[o][:, c0:c0 + cw], ps[:, 0:cw])

        head_state = {}

        def emit_head_scores(h):
            g = h // 2
            r0 = (h % 2) * D_HEAD          # row offset of head h inside tile g
            # S^T diagonal strips into ONE psum bank [128, 512]. The first
            # matmul carries start=True (marks the whole bank pending), later
            # ones first-touch-overwrite their regions, the last carries stop.
            s_ps = psum.tile([128, STRIP_COLS], F32, tag="s", name="sS", bufs=2)
            mm = nc.tensor.matmul
            for u in range(NSTRIP):
                mm(s_ps[:, 64 * u:64 * u + 64],
                   lhsT=kt[g][r0:r0 + D_HEAD, 64 * (u + 1):64 * (u + 1) + 128],
                   rhs=qt[g][r0:r0 + D_HEAD, 64 * u:64 * u + 64],
                   start=(u == 0), stop=(u == NSTRIP - 1), skip_group_check=True)
            # P^T = exp(SCALE * S^T); invalid entries hold finite junk
            # (|SCALE*s| <~ 12, no bf16 overflow), zeroed by the mask below.
            pt_t = work.tile([128, STRIP_COLS], BF16, tag="p", bufs=6, name=f"pt{h}")
            # qt/kt carry 32x-scaled values (host pre-scales wqkv by 32), so
            # the psum holds 1024*S; 1/1024 folds exactly into the exp scale.
            nc.scalar.activation(pt_t[:, 0:256], s_ps[:, 0:256],
                                 mybir.ActivationFunctionType.Exp,
                                 bias=0.0, scale=float(SCALE / 1024.0))
            nc.scalar.activation(pt_t[:, 256:512], s_ps[:, 256:512],
                                 mybir.ActivationFunctionType.Exp,
                                 bias=0.0, scale=float(SCALE / 1024.0))
            # zero the out-of-band entries (Pool; otherwise idle here).
            # Two halves, each pipelined behind its exp, to shorten the
            # exp->mask->PV round trip.
            nc.gpsimd.tensor_mul(pt_t[:, 0:256], pt_t[:, 0:256], mT[:, 0:256])
            nc.gpsimd.tensor_mul(pt_t[:, 256:512], pt_t[:, 256:512], mT[:, 256:512])
            head_state[h] = pt_t

        def emit_head_pv(h):
            g = h // 2
            r0 = (h % 2) * D_HEAD
            pt_t = head_state.pop(h)
            # out'_h [128, 512 q]: rows 0:64 = out_h^T, rows 64:128 = softmax
            # denominators (from the V' ones-blocks). Each strip u is a single
            # matmul over its 128-key window: odd u hits an aligned A-tile,
            # even u the 64-skewed B-tile.
            op = psum.tile([128, NLOC], F32, tag="pA", name="opsum", bufs=2)
            mm = nc.tensor.matmul
            for u in range(NSTRIP):
                vtile = vt[(u + 1) // 2] if u % 2 == 1 else vtB[u // 2]
                mm(op[:, 64 * u:64 * u + 64],
                   lhsT=vtile[:, h * 128:(h + 1) * 128],
                   rhs=pt_t[:, 64 * u:64 * u + 64],
                   start=(u == 0), stop=(u == NSTRIP - 1), skip_group_check=True)
            # normalize fused into the psum->sbuf copy: ao = out * (1/denom)
            # (DVE divide is rejected by the BIR verifier - no divide ALU;
            # Pool cannot read PSUM; ACT-copy decoupling adds a second psum
            # reader engine whose WAR semaphores cost more than it saves)
            rbb = work.tile([D_HEAD, NLOC], F32, tag="rbb", bufs=2, name="rbb")
            nc.vector.reciprocal(rbb, op[D_HEAD:128, :])
            nc.vector.tensor_mul(ao[g][r0:r0 + D_HEAD, :], op[0:D_HEAD, :], rbb)

        for o in range(DT):
            emit_qk(o)
            if o >= 3:
                emit_head_pv(2 * (o - 2))
                emit_head_pv(2 * (o - 2) + 1)
            if o >= 2:
                emit_head_scores(2 * (o - 1))
                emit_head_scores(2 * (o - 1) + 1)
        # tail: pair 0 (ancient tiles) and pair 7; PE filler (pv 12/13, dmy)
        # covers the exp->mask round trips of the last-scored pairs.
        emit_head_scores(0)
        emit_head_scores(1)
        emit_head_scores(14)
        emit_head_scores(15)
        emit_head_pv(12)
        emit_head_pv(13)
        # 8 trivial matmuls make PE observe every wo DMA queue semaphore
        # here (satisfied by now - wo was prefetched), so phase C's matmuls
        # don't each need a DMA wait slot (HW limit: 2 sync waits per matmul)
        dmy = psum.tile([1, 1], F32, tag="pt", name="dmy", bufs=2)
        for g in range(DT):
            nc.tensor.matmul(dmy, lhsT=wo[g][0:1, 0:1],
                             rhs=wo[g][0:1, 0:1],
                             start=(g == 0), stop=(g == DT - 1))
        emit_head_pv(0)
        emit_head_pv(1)
        emit_head_pv(14)
        emit_head_pv(15)

        # ---- Phase C: out = attnout @ wout^T (+ b_out); g=0 contracted last
        # so the chains only need ao[0] (heads 0/1, finishing on DVE) at the
        # very end of each chain ----
        gorder = [1, 2, 3, 4, 5, 6, 7, 0]

        chain_idx = [0]

        def outproj_chain(n, c0, cw, ps=None, cp=None, dma_eng=None):
            if ps is None:
                ps = next_ps(cw, "psc")
            if cp is None:
                cp = copy_dve
            for gi, g in enumerate(gorder):
                nc.tensor.matmul(
                    ps, lhsT=ao[g][:, n * 128:(n + 1) * 128],
                    rhs=wo[g][:, c0:c0 + cw],
                    start=(gi == 0), stop=(gi == DT - 1 and not use_bout))
            if use_bout:
                nc.tensor.matmul(
                    ps, lhsT=ones[0:1, 0:128],
                    rhs=bout_sb[0:1, c0:c0 + cw],
                    start=False, stop=True)
            # y is written in bf16 (host casts back to fp32): halves the DMA
            # bytes and doubles the DVE copy rate on the drain path.
            ot = outp.tile([128, cw], BF16, tag="out", name=f"ot{cw}")
            cp(ot, ps)
            if dma_eng is None:
                # alternate the y DMAs between the SP and Pool queues so the
                # drain's transfers overlap instead of serializing on SP
                dma_eng = nc.sync if chain_idx[0] % 2 == 0 else nc.gpsimd
            chain_idx[0] += 1
            dma_eng.dma_start(out=y[n * 128:(n + 1) * 128, c0:c0 + cw], in_=ot)

        for n in range(NQB):
            for oh in range(2):
                if n == NQB - 1 and oh == 1:
                    # drain choreography: the 384's DVE copy + ACT-queue DMA
                    # and the final 128's ACT copy (psum tag "s" is ACT-read)
                    # + SP DMA run on disjoint engine/queue pairs, so the two
                    # last completions overlap instead of serializing.
                    outproj_chain(n, 512, 384, dma_eng=nc.scalar)
                    fps = psum.tile([128, 128], F32, tag="s", name="fin", bufs=2)
                    outproj_chain(n, 896, 128, ps=fps, cp=copy_act,
                                  dma_eng=nc.sync)
                else:
                    outproj_chain(n, oh * 512, 512)

    return nc


_PROG_CACHE: dict = {}


def _get_program(use_bqkv: bool, use_bout: bool) -> bass.Bass:
    key = (use_bqkv, use_bout)
    if key not in _PROG_CACHE:
        nc = _emit_program(use_bqkv, use_bout)
        if not nc.is_finalized():
            nc.finalize()
        _PROG_CACHE[key] = nc
    return _PROG_CACHE[key]


def _build_maskT(core0: bool) -> np.ndarray:
    """0/1 valid bits for the 8 diagonal S^T strips, [128 k-rows, 512 cols].

    Strip u holds keys 64(u+1)+k' vs queries 64u+q'; valid iff
    1 <= k'-q' <= 64. On core 0 the u=0 strip's keys 64:128 (k' < 64) are
    x-padding, so they are masked off too."""
    m = np.zeros((128, MASKT_COLS), np.float32)
    kp = np.arange(128)[:, None]
    qp = np.arange(64)[None, :]
    band = (kp - qp >= 1) & (kp - qp <= WINDOW)
    for u in range(NSTRIP):
        v = band if not (core0 and u == 0) else band & (kp >= 64)
        m[:, 64 * u:64 * u + 64] = v
    m[:, STRIP_COLS:STRIP_COLS + 128] = np.eye(128, dtype=np.float32)
    return m


def _split8(a):
    """fp8 hi/lo split of an fp32 array: a ~= hi + lo (both e4m3, unscaled).

    Callers pre-scale the array so its values are ~N(0,1); the lo residual
    (~2^-4 of the value) then sits in e4m3's normal range."""
    hi = a.astype(NP_FP8)
    lo = (a - hi.astype(np.float32)).astype(NP_FP8)
    return hi, lo


def _pairA(hT):
    """[1024, C] -> [512, 2C]: row 128j+p = [chunk(2j) row p | chunk(2j+1) row p]."""
    C = hT.shape[1]
    t = hT.reshape(4, 2, 128, C).transpose(0, 2, 1, 3)
    return np.ascontiguousarray(t.reshape(512, 2 * C))


def kernel(x, w_qkv, b_qkv, w_out, b_out):
    global LAST_RESULT
    x = np.asarray(x, dtype=np.float32)
    w_qkv = np.asarray(w_qkv, dtype=np.float32)
    b_qkv = np.asarray(b_qkv, dtype=np.float32)
    w_out = np.asarray(w_out, dtype=np.float32)
    b_out = np.asarray(b_out, dtype=np.float32)

    B = x.shape[0]
    assert x.shape == (1, N_SEQ, D_MODEL), x.shape
    xa = x[0]

    use_bqkv = bool(np.any(b_qkv))
    use_bout = bool(np.any(b_out))
    nc = _get_program(use_bqkv, use_bout)

    # wqkv pre-scaled by 32 so w_lo is fp8-normal-range; the 1024x on the
    # scores psum folds into the exp scale, and the 32x on V folds into
    # wout/32 (exact exponent shifts; ao just carries 32x in bf16).
    wqkvT = np.ascontiguousarray(w_qkv.T) * 32.0             # [1024, 3072] f32
    w_hi, w_lo = _split8(wqkvT)
    wqkvA = np.concatenate(
        [_pairA(w_hi[:, p * D_MODEL:(p + 1) * D_MODEL]) for p in range(3)], axis=1)
    wqkvL = np.concatenate(
        [_pairA(w_lo[:, p * D_MODEL:(p + 1) * D_MODEL]) for p in range(3)], axis=1)
    woutT = np.ascontiguousarray(w_out.T / 32.0).astype(NP_BF16)  # [1024, 1024]
    mT_std = _build_maskT(core0=False).astype(NP_BF16)
    mT_first = _build_maskT(core0=True).astype(NP_BF16)

    in_maps = []
    for c in range(N_CORES):
        s = c * NLOC
        if c == 0:
            blk = np.concatenate([np.zeros((HALO, D_MODEL), np.float32), xa[0:NLOC]], axis=0)
        else:
            blk = xa[s - HALO:s + NLOC]
        xTblk = np.ascontiguousarray(blk.T)                 # [1024, 640] f32
        x_hi, x_lo = _split8(xTblk)
        im = {
            "xA": _pairA(x_hi),
            "xL": _pairA(x_lo),
            "wqkvA": wqkvA,
            "wqkvL": wqkvL,
            "woutT": woutT,
            "maskT": mT_first if c == 0 else mT_std,
        }
        if use_bqkv:
            im["bqkv"] = b_qkv.reshape(1, 3 * D_MODEL).astype(NP_BF16)
        if use_bout:
            im["bout"] = b_out.reshape(1, D_MODEL).astype(NP_BF16)
        in_maps.append(im)

    res = run_bass_kernel_spmd(nc, in_maps, list(range(N_CORES)))
    LAST_RESULT = res
    out = np.concatenate(
        [res.results[c]["y"].astype(np.float32) for c in range(N_CORES)], axis=0)
    return out.reshape(B, N_SEQ, D_MODEL)

